# revision 1
# baseline (speedup 1.0000x reference)
"""Trainium2 Bass kernel for nn_Decoder (LSTM decoder w/ attention).

Sharding: 8-way model parallel over hidden dim D for the recurrence
(each core owns 128 of 1024 dims = all 4 gates for those dims), vocab
shard (4000 rows/core) for the output projection, which runs as a
batched matmul over all T*B rows interleaved with the recurrence.

All matmul operands are bf16 (1 cycle/row on the PE vs 4 for fp32);
accumulation stays fp32 in PSUM, LSTM cell state and softmax stay fp32.
Collective payloads and the final score store are bf16 (host casts back).
"""

import numpy as np
import ml_dtypes
import bass_rust
import concourse.bass as bass  # noqa: F401  (bass types used via bacc)
import concourse.tile as tile
from concourse import bacc, mybir
from concourse.bass_utils import run_bass_kernel_spmd
from concourse.masks import make_identity

V, E, D = 32000, 512, 1024
TWO_E = 1024
B, S, T = 32, 64, 48
P = 8
DSH = D // P        # 128 hidden dims per core
VSH = V // P        # 4000 vocab rows per core
FP = mybir.dt.float32
BF = mybir.dt.bfloat16
NBF = ml_dtypes.bfloat16
AF = mybir.ActivationFunctionType
RG = [list(range(P))]
X = mybir.AxisListType.X

# gates0 lhsT layout: [h2 (8x128) | word (4x128) | ones/bias (128) | av (8x128)]
NK0 = 21
# gates1 lhsT layout: [h1 (8x128) | ones/bias (128)]
NK1 = 9


def _build(t_steps=T):
    nc = bacc.Bacc("TRN2", target_bir_lowering=False, debug=False, num_devices=P)
    CW = t_steps * 32  # avhist block width (cols = t*32+b)

    w0s_p = nc.declare_dram_parameter("w0s", [128, NK0 * 512], BF, isOutput=False)
    w1s_p = nc.declare_dram_parameter("w1s", [128, NK1 * 512], BF, isOutput=False)
    wcs_p = nc.declare_dram_parameter("wcs", [128, 16 * 128], BF, isOutput=False)
    wot_p = nc.declare_dram_parameter("wot", [1024, VSH], BF, isOutput=False)
    wpt_p = nc.declare_dram_parameter("wpt", [128, 8 * 128], BF, isOutput=False)
    enct_p = nc.declare_dram_parameter("enct", [1024, 2048], BF, isOutput=False)
    encse_p = nc.declare_dram_parameter("encse", [64, 32 * 128], BF, isOutput=False)
    wordt_p = nc.declare_dram_parameter("wordt", [128, t_steps * 128], BF,
                                        isOutput=False)
    h0t_p = nc.declare_dram_parameter("h0t", [128, 8 * 32], BF, isOutput=False)
    c0s_p = nc.declare_dram_parameter("c0s", [32, 128], FP, isOutput=False)
    # scores stored transposed: [vocab_shard, t*32+b]
    out_p = nc.declare_dram_parameter("out", [VSH, CW], BF, isOutput=True)

    with tile.TileContext(nc) as tc:
        with (
            tc.tile_pool(name="res", bufs=1) as res,
            tc.tile_pool(name="wk", bufs=2) as wk,
            tc.tile_pool(name="ps1", bufs=1, space="PSUM") as ps1,
            tc.tile_pool(name="ps2", bufs=2, space="PSUM") as ps2,
            tc.tile_pool(name="dr", bufs=2, space="DRAM") as dr,
        ):
            # ---- resident SBUF ----
            w0s = res.tile([128, NK0 * 512], BF, tag="w0s")
            w1s = res.tile([128, NK1 * 512], BF, tag="w1s")
            wcs = res.tile([128, 16 * 128], BF, tag="wcs")
            at = res.tile([128, 2048], BF, tag="at")
            encse = res.tile([64, 32 * 128], BF, tag="encse")
            avhist = res.tile([128, 8 * CW], BF, tag="avhist")
            wotsb = res.tile([128, 8 * VSH], BF, tag="wotsb")
            wordsb = res.tile([128, t_steps * 128], BF, tag="wordsb")
            h1full = res.tile([128, 8 * 32], BF, tag="h1full")
            h2full = res.tile([128, 8 * 32], BF, tag="h2full")
            c = res.tile([32, 128], FP, tag="c")
            ones = res.tile([128, 32], BF, tag="ones")
            id32 = res.tile([32, 32], FP, tag="id32")
            id32b = res.tile([32, 32], BF, tag="id32b")
            wpt = res.tile([128, 8 * 128], BF, tag="wpt")

            # ---- init loads (split for overlap) ----
            for kk in range(NK0):
                nc.sync.dma_start(out=w0s[:, 512 * kk:512 * (kk + 1)],
                                  in_=w0s_p[:, 512 * kk:512 * (kk + 1)])
            for kk in range(NK1):
                nc.sync.dma_start(out=w1s[:, 512 * kk:512 * (kk + 1)],
                                  in_=w1s_p[:, 512 * kk:512 * (kk + 1)])
            nc.sync.dma_start(out=wcs[:], in_=wcs_p[:])
            nc.sync.dma_start(out=encse[:], in_=encse_p[:])
            nc.sync.dma_start(out=h2full[:], in_=h0t_p[:])
            nc.sync.dma_start(out=c[:], in_=c0s_p[:])
            nc.sync.dma_start(out=wpt[:], in_=wpt_p[:])
            nc.sync.dma_start(out=wordsb[:], in_=wordt_p[:])
            for j in range(8):
                nc.sync.dma_start(
                    out=wotsb[:, VSH * j:VSH * (j + 1)],
                    in_=wot_p[128 * j:128 * (j + 1), :])

            nc.vector.memset(ones[:], 0.0)
            nc.vector.memset(ones[0:1, :], 1.0)
            make_identity(nc, id32[:])
            make_identity(nc, id32b[:])

            # ---- attention scores AT_shard = Wp_shard @ encT ----
            at_ps = [
                ps2.tile([128, 512], FP, tag="mm", name="atps_0"),
                ps2.tile([128, 512], FP, tag="mm", name="atps_1"),
                ps2.tile([128, 512], FP, tag="tr", name="atps_2"),
                ps1.tile([128, 512], FP, tag="av", name="atps_3"),
            ]
            for kk in range(8):
                et = wk.tile([128, 2048], BF, tag="enct", bufs=1)
                nc.sync.dma_start(out=et[:], in_=enct_p[128 * kk:128 * (kk + 1), :])
                for nch in range(4):
                    nc.tensor.matmul(at_ps[nch][:],
                                     wpt[:, 128 * kk:128 * (kk + 1)],
                                     et[:, 512 * nch:512 * (nch + 1)],
                                     start=(kk == 0), stop=(kk == 7))
            for nch in range(4):
                nc.scalar.activation(at[:, 512 * nch:512 * (nch + 1)],
                                     at_ps[nch][:], AF.Copy)

            # output projection (transposed): outT[v, (t,b)] += WoT.T @ av
            def _emit_pb(n, vt, width):
                base = 512 * n
                mv = min(128, VSH - 128 * vt)
                bp = ps2.tile([mv, width], FP, tag="g0", name=f"pb_{n}_{vt}")
                for j in range(8):
                    nc.tensor.matmul(
                        bp[:], wotsb[:, VSH * j + 128 * vt:VSH * j + 128 * vt + mv],
                        avhist[:, j * CW + base:j * CW + base + width],
                        start=(j == 0), stop=(j == 7))
                bs_ = wk.tile([mv, width], BF, tag="bstg", name=f"pbs_{n}_{vt}")
                nc.vector.tensor_copy(bs_[:], bp[:])
                nc.sync.dma_start(
                    out=out_p[128 * vt:128 * vt + mv, base:base + width],
                    in_=bs_[:])

            # ---- recurrence ----
            for t in range(t_steps):
                # gates0: g0 = W0 @ [h2; word; 1; av]
                g0 = ps2.tile([32, 512], FP, tag="g0")
                mms = []
                for j in range(8):
                    mms.append((h2full[:, 32 * j:32 * (j + 1)], j))
                for j in range(4):
                    mms.append((wordsb[:, 128 * t + 32 * j:128 * t + 32 * (j + 1)],
                                8 + j))
                mms.append((ones[:], 12))
                if t > 0:
                    for j in range(8):
                        mms.append((avhist[:, j * CW + 32 * (t - 1):
                                           j * CW + 32 * t], 13 + j))
                for i, (lhsT, kk) in enumerate(mms):
                    nc.tensor.matmul(g0[:], lhsT,
                                     w0s[:, 512 * kk:512 * (kk + 1)],
                                     start=(i == 0), stop=(i == len(mms) - 1))

                # projection chunk here fills the h1-AllGather wait on the PE
                if t_steps == 48 and 16 <= t < 48:
                    n = (t - 16) // 16
                    _emit_pb(n, 2 * ((t - 16) % 16), 512)

                # lstm cell 0 (gate order i,f,o,g along free dim)
                sifo = wk.tile([32, 384], FP, tag="sifo")
                tg = wk.tile([32, 128], FP, tag="tg")
                nc.scalar.activation(sifo[:], g0[:, 0:384], AF.Sigmoid)
                nc.scalar.activation(tg[:], g0[:, 384:512], AF.Tanh)
                t1 = wk.tile([32, 128], FP, tag="t1")
                t2 = wk.tile([32, 128], FP, tag="t2")
                nc.vector.tensor_mul(t1[:], sifo[:, 128:256], c[:])
                nc.vector.tensor_mul(t2[:], sifo[:, 0:128], tg[:])
                nc.vector.tensor_add(c[:], t1[:], t2[:])
                tc1 = wk.tile([32, 128], FP, tag="tc1")
                nc.scalar.activation(tc1[:], c[:], AF.Tanh)
                h1 = wk.tile([32, 128], FP, tag="h1")
                nc.vector.tensor_mul(h1[:], sifo[:, 256:384], tc1[:])

                # h1 -> h1T shard, AllGather -> h1full
                trp = ps2.tile([128, 32], FP, tag="tr")
                nc.tensor.transpose(trp[:], h1[:], id32[:])
                h1t = wk.tile([128, 32], BF, tag="h1t")
                nc.scalar.activation(h1t[:], trp[:], AF.Copy)
                b1 = dr.tile([128, 32], BF, tag="b1")
                o1 = dr.tile([1024, 32], BF, tag="o1")
                nc.sync.dma_start(out=b1[:], in_=h1t[:])
                nc.gpsimd.collective_compute(
                    "AllGather", mybir.AluOpType.bypass,
                    replica_groups=RG, ins=[b1.opt()], outs=[o1.opt()])
                nc.sync.dma_start(
                    out=h1full[:].rearrange("p (j b) -> p j b", b=32),
                    in_=o1[:].rearrange("(j p) b -> p j b", p=128))

                # gates1: g1 = W1 @ [h1; 1]
                g1 = ps2.tile([32, 512], FP, tag="mm")
                for j in range(8):
                    nc.tensor.matmul(g1[:], h1full[:, 32 * j:32 * (j + 1)],
                                     w1s[:, 512 * j:512 * (j + 1)],
                                     start=(j == 0), stop=False)
                nc.tensor.matmul(g1[:], ones[:], w1s[:, 512 * 8:512 * 9],
                                 start=False, stop=True)

                # lstm cell 1 (same c state threaded; gate order i,f,o,g)
                sifo2 = wk.tile([32, 384], FP, tag="sifo")
                tg2 = wk.tile([32, 128], FP, tag="tg")
                nc.scalar.activation(sifo2[:], g1[:, 0:384], AF.Sigmoid)
                nc.scalar.activation(tg2[:], g1[:, 384:512], AF.Tanh)
                nc.vector.tensor_mul(t1[:], sifo2[:, 128:256], c[:])
                nc.vector.tensor_mul(t2[:], sifo2[:, 0:128], tg2[:])
                nc.vector.tensor_add(c[:], t1[:], t2[:])
                nc.scalar.activation(tc1[:], c[:], AF.Tanh)
                h2 = wk.tile([32, 128], FP, tag="h2")
                nc.vector.tensor_mul(h2[:], sifo2[:, 256:384], tc1[:])

                # h2 -> h2T shard
                trp2 = ps2.tile([128, 32], FP, tag="tr")
                nc.tensor.transpose(trp2[:], h2[:], id32[:])
                h2t = wk.tile([128, 32], BF, tag="h2t")
                nc.scalar.activation(h2t[:], trp2[:], AF.Copy)

                # logits partial per-batch (transposed): lgT[:, b] = at_b.T @ h2t[:, b]
                lgps = ps2.tile([64, 32], FP, tag="mm", name="lgps")
                for b in range(32):
                    nc.tensor.matmul(lgps[:, b:b + 1],
                                     at[:, 64 * b:64 * (b + 1)],
                                     h2t[:, b:b + 1],
                                     start=True, stop=True)
                lgc = wk.tile([64, 32], BF, tag="lgc")
                nc.vector.tensor_copy(lgc[:], lgps[:])

                # one merged AllGather: [h2T shard (128x32) | logits partial (2048)]
                bm = dr.tile([192, 32], BF, tag="bm")
                om = dr.tile([1536, 32], BF, tag="om")
                nc.sync.dma_start(out=bm[0:128, :], in_=h2t[:])
                bm_ap = bm[:]
                # flat bm index for logit (b, s) is 128*32 + 64b + s; lgc is
                # [s, b] so iterate (s, b): strides (1, 64)
                bm_lp = bass_rust.AP(bm_ap.tensor, bm_ap.offset + 128 * 32,
                                     [[1, 64], [64, 32]])
                nc.sync.dma_start(out=bm_lp, in_=lgc[:])

                # projection chunk here fills the h2/logits-AllGather wait
                if t_steps == 48 and 16 <= t < 48:
                    n = (t - 16) // 16
                    _emit_pb(n, 2 * ((t - 16) % 16) + 1, 512)
                nc.gpsimd.collective_compute(
                    "AllGather", mybir.AluOpType.bypass,
                    replica_groups=RG, ins=[bm.opt()], outs=[om.opt()])
                om_ap = om[:]
                # h2full[p, j*32+b] = om[j*192 + p, b]
                nc.sync.dma_start(
                    out=h2full[:].rearrange("p (j b) -> p j b", b=32),
                    in_=bass_rust.AP(om_ap.tensor, om_ap.offset,
                                     [[32, 128], [192 * 32, 8], [1, 32]]))
                # ls[b, r*64+s] = om[r*192 + 128 + (64b+s)//32, (64b+s)%32]
                ls = wk.tile([32, 8 * 64], BF, tag="ls")
                nc.sync.dma_start(
                    out=ls[:].rearrange("p (r s) -> p r s", s=64),
                    in_=bass_rust.AP(om_ap.tensor, om_ap.offset + 128 * 32,
                                     [[64, 32], [192 * 32, 8], [1, 64]]))
                lg0 = wk.tile([32, 64], FP, tag="lgs")
                nc.vector.tensor_add(lg0[:], ls[:, 0:64], ls[:, 64:128])
                nc.vector.tensor_add(lg0[:], lg0[:], ls[:, 128:192])
                nc.vector.tensor_add(lg0[:], lg0[:], ls[:, 192:256])
                nc.vector.tensor_add(lg0[:], lg0[:], ls[:, 256:320])
                nc.vector.tensor_add(lg0[:], lg0[:], ls[:, 320:384])
                nc.vector.tensor_add(lg0[:], lg0[:], ls[:, 384:448])
                nc.vector.tensor_add(lg0[:], lg0[:], ls[:, 448:512])

                # softmax over s
                mx = wk.tile([32, 1], FP, tag="mx")
                nc.vector.reduce_max(mx[:], lg0[:], axis=X, negate=True)
                ex = wk.tile([32, 64], FP, tag="ex")
                nc.scalar.activation(ex[:], lg0[:], AF.Exp, bias=mx[:])
                sm = wk.tile([32, 1], FP, tag="sm")
                nc.vector.reduce_sum(sm[:], ex[:], axis=X)
                rc = wk.tile([32, 1], FP, tag="rc")
                nc.vector.reciprocal(rc[:], sm[:])
                al = wk.tile([32, 64], FP, tag="al")
                nc.vector.tensor_scalar_mul(al[:], ex[:], rc[:])

                # alpha -> alphaT
                trp3 = ps2.tile([64, 32], FP, tag="tr")
                nc.tensor.transpose(trp3[:], al[:], id32[:])
                alt = wk.tile([64, 32], BF, tag="alt")
                nc.scalar.activation(alt[:], trp3[:], AF.Copy)

                # context per-batch (transposed): cxt[:, b] = encse_b.T @ alpha_b
                cxt_ps = ps2.tile([128, 32], FP, tag="tr")
                for b in range(32):
                    nc.tensor.matmul(cxt_ps[:, b:b + 1],
                                     encse[:, 128 * b:128 * (b + 1)],
                                     alt[:, b:b + 1],
                                     start=True, stop=True)
                cxt = wk.tile([128, 32], BF, tag="cxt")
                nc.scalar.activation(cxt[:], cxt_ps[:], AF.Copy)

                # combine partial: av_preT[m-dims, b] over own 256 K dims
                avp = ps1.tile([128, 256], FP, tag="av")
                for m in range(8):
                    nc.tensor.matmul(avp[:, 32 * m:32 * (m + 1)],
                                     wcs[:, (0 * 8 + m) * 128:(0 * 8 + m) * 128 + 128],
                                     h2t[:], start=True, stop=False)
                    nc.tensor.matmul(avp[:, 32 * m:32 * (m + 1)],
                                     wcs[:, (1 * 8 + m) * 128:(1 * 8 + m) * 128 + 128],
                                     cxt[:], start=False, stop=True)
                avs = wk.tile([128, 256], BF, tag="avs")
                nc.vector.tensor_copy(avs[:], avp[:])
                bav = dr.tile([1024, 32], BF, tag="bav")
                oav = dr.tile([1024, 32], BF, tag="oav")
                nc.sync.dma_start(
                    out=bav[:].rearrange("(m p) b -> p m b", p=128),
                    in_=avs[:].rearrange("p (m b) -> p m b", b=32))
                nc.gpsimd.collective_compute(
                    "AllReduce", mybir.AluOpType.add,
                    replica_groups=RG, ins=[bav.opt()], outs=[oav.opt()])
                avpre = wk.tile([128, 256], BF, tag="avpre")
                nc.sync.dma_start(
                    out=avpre[:].rearrange("p (j b) -> p j b", b=32),
                    in_=oav[:].rearrange("(j p) b -> p j b", p=128))
                # av = tanh(av_pre), written into avhist column group t
                dst = avhist[:].rearrange("p (j c) -> p j c", c=CW)[:, :, 32 * t:32 * (t + 1)]
                nc.scalar.activation(dst, avpre[:].rearrange("p (j b) -> p j b", b=32),
                                     AF.Tanh)


            # ---- remaining output-projection chunks ----
            if t_steps == 48:
                for vt in range(32):
                    _emit_pb(2, vt, 512)
            else:
                nch = (CW + 511) // 512
                for n in range(nch):
                    w = min(512, CW - 512 * n)
                    for vt in range(32):
                        _emit_pb(n, vt, w)

    nc.compile()
    return nc


def _prep(inputs, t_steps=T):
    g = {k: np.asarray(v) for k, v in inputs.items()}
    src = g["src_encodings"].astype(np.float32)          # [S, B, 2E]
    h0 = g["h0"].astype(np.float32)
    c0 = g["c0"].astype(np.float32)
    emb = g["embedding"].astype(np.float32)
    Wp = g["W_proj"].astype(np.float32)
    Wc = g["W_combine"].astype(np.float32)
    Wo = g["W_out"].astype(np.float32)
    Wih0 = g["W_ih0"].astype(np.float32)
    Whh0 = g["W_hh0"].astype(np.float32)
    bih0 = g["b_ih0"].astype(np.float32)
    bhh0 = g["b_hh0"].astype(np.float32)
    Wih1 = g["W_ih1"].astype(np.float32)
    Whh1 = g["W_hh1"].astype(np.float32)
    bih1 = g["b_ih1"].astype(np.float32)
    bhh1 = g["b_hh1"].astype(np.float32)
    tgt = np.asarray(g["tgt_tensor"]).astype(np.int64)   # [T, B]

    W1 = Wih1 + Whh1
    b0 = bih0 + bhh0
    b1 = bih1 + bhh1

    # shared across cores
    wemb = emb[tgt[:t_steps]]                            # [t, B, E]
    # wordt: [128, t*128]; step block t = wordT[:,t] split into 4 j-blocks
    wordt = (wemb.transpose(0, 2, 1)                     # [t, E, B]
             .reshape(t_steps, 4, 128, 32)
             .transpose(2, 0, 1, 3).reshape(128, t_steps * 128))
    wordt = np.ascontiguousarray(wordt).astype(NBF)
    enct = np.ascontiguousarray(
        src.transpose(2, 1, 0).reshape(1024, 2048)).astype(NBF)  # [e, b*64+s]
    h0t = np.ascontiguousarray(
        h0.T.reshape(8, 128, 32).transpose(1, 0, 2).reshape(128, 256)).astype(NBF)

    in_maps = []
    for k in range(P):
        rows = np.concatenate([gg * 1024 + k * 128 + np.arange(128)
                               for gg in (0, 1, 3, 2)])  # [i|f|o|g] x 128 dims
        # W0sT_aug rows: [h2 1024 | word 512 | bias 1 | pad | av 1024]
        w0a = np.zeros((NK0 * 128, 512), np.float32)
        w0a[0:1024] = Whh0[rows].T
        w0a[1024:1536] = Wih0[rows, 0:512].T
        w0a[1536] = b0[rows]
        w0a[1664:2688] = Wih0[rows, 512:1536].T
        w0s = np.ascontiguousarray(
            w0a.reshape(NK0, 128, 512).transpose(1, 0, 2)
            .reshape(128, NK0 * 512)).astype(NBF)

        w1a = np.zeros((NK1 * 128, 512), np.float32)
        w1a[0:1024] = W1[rows].T
        w1a[1024] = b1[rows]
        w1s = np.ascontiguousarray(
            w1a.reshape(NK1, 128, 512).transpose(1, 0, 2)
            .reshape(128, NK1 * 512)).astype(NBF)

        # Wc own-K slice: h dims [128k..] and ctx dims [1024+128k..]
        hs = slice(k * 128, k * 128 + 128)
        cs = slice(1024 + k * 128, 1024 + k * 128 + 128)
        wc_own = np.concatenate([Wc[:, hs], Wc[:, cs]], axis=1)  # [1024, 256]
        blocks = []
        for j in range(2):
            for m in range(8):
                blocks.append(wc_own[128 * m:128 * (m + 1),
                                     128 * j:128 * (j + 1)].T)
        wcs = np.ascontiguousarray(np.concatenate(blocks, axis=1)).astype(NBF)

        wot = np.ascontiguousarray(Wo[VSH * k:VSH * (k + 1)].T).astype(NBF)
        wpt_ = Wp[128 * k:128 * (k + 1), :].T                       # [1024, 128]
        wpt = np.ascontiguousarray(
            wpt_.reshape(8, 128, 128).transpose(1, 0, 2)
            .reshape(128, 8 * 128)).astype(NBF)
        # encse2[s, b*128 + e] = src[s, b, e_shard]
        encse = np.ascontiguousarray(
            src[:, :, 128 * k:128 * (k + 1)].reshape(64, 32 * 128)).astype(NBF)
        c0s = np.ascontiguousarray(c0[:, 128 * k:128 * (k + 1)])

        in_maps.append({
            "w0s": w0s, "w1s": w1s, "wcs": wcs, "wot": wot, "wpt": wpt,
            "enct": enct, "encse": encse, "wordt": wordt,
            "h0t": h0t, "c0s": c0s,
        })
    return in_maps


_CACHE = {}


def _get_nc(t_steps=T):
    if t_steps not in _CACHE:
        _CACHE[t_steps] = _build(t_steps)
    return _CACHE[t_steps]


def run_device(inputs, trace=False, t_steps=T):
    nc = _get_nc(t_steps)
    in_maps = _prep(inputs, t_steps)
    return run_bass_kernel_spmd(nc, in_maps, core_ids=list(range(P)), trace=trace)


def assemble(results, t_steps=T):
    return np.concatenate(
        [np.ascontiguousarray(np.asarray(results[k]["out"]).astype(np.float32).T)
         .reshape(t_steps, B, VSH) for k in range(P)],
        axis=2)


def kernel(**inputs):
    r = run_device(inputs)
    return assemble(r.results)



# revision 9
# speedup vs baseline: 1.5017x; 1.5017x over previous
"""Trainium2 Bass kernel for nn_Decoder (LSTM decoder w/ attention).

Sharding: 8-way model parallel over hidden dim D for the recurrence
(each core owns 128 of 1024 dims = all 4 gates for those dims), vocab
shard (4000 rows/core) for the output projection, which runs as a
batched matmul over all T*B rows interleaved with the recurrence.

All matmul operands are bf16 (1 cycle/row on the PE vs 4 for fp32);
accumulation stays fp32 in PSUM, LSTM cell state and softmax stay fp32.
Collective payloads and the final score store are bf16 (host casts back).

Perf-critical details (learned from NTFF traces):
- Collective input stagings must be contiguous >=32B segments: sub-32B
  scattered HBM writes trigger read-modify-write and delay the DMA
  completion semaphore by ~20us, stalling the next collective trigger.
  Logits are therefore computed directly in [b, s] orientation and the
  AllReduce buffers keep the natural [128, 256] layout.
- Projection chunks (width 256) are interleaved into all three
  per-step collective flights to keep the PE busy (p-state) and to
  absorb the ~24us/step of collective latency.
- Projection stores issue from the Scalar HWDGE queue so they never
  head-of-line block the Sync queue carrying recurrence DMAs.
"""

import numpy as np
import ml_dtypes
import bass_rust
import concourse.bass as bass  # noqa: F401  (bass types used via bacc)
import concourse.tile as tile
from concourse import bacc, mybir
from concourse.bass_utils import run_bass_kernel_spmd
from concourse.masks import make_identity

V, E, D = 32000, 512, 1024
TWO_E = 1024
B, S, T = 32, 64, 48
P = 8
DSH = D // P        # 128 hidden dims per core
VSH = V // P        # 4000 vocab rows per core
FP = mybir.dt.float32
BF = mybir.dt.bfloat16
NBF = ml_dtypes.bfloat16
AF = mybir.ActivationFunctionType
RG = [list(range(P))]
X = mybir.AxisListType.X

# gates0 lhsT layout: [h2 (8x128) | word (4x128) | ones/bias (128) | av (8x128)]
NK0 = 21
# gates1 lhsT layout: [h1 (8x128) | ones/bias (128)]
NK1 = 9

PW = 256            # projection chunk width (cols = t*32+b)


def _build(t_steps=T):
    nc = bacc.Bacc("TRN2", target_bir_lowering=False, debug=False, num_devices=P)
    CW = t_steps * 32  # avhist block width (cols = t*32+b)

    w0s_p = nc.declare_dram_parameter("w0s", [128, NK0 * 512], BF, isOutput=False)
    w1s_p = nc.declare_dram_parameter("w1s", [128, NK1 * 512], BF, isOutput=False)
    wcs_p = nc.declare_dram_parameter("wcs", [128, 16 * 128], BF, isOutput=False)
    wot_p = nc.declare_dram_parameter("wot", [1024, VSH], BF, isOutput=False)
    wpt_p = nc.declare_dram_parameter("wpt", [128, 8 * 128], BF, isOutput=False)
    enct_p = nc.declare_dram_parameter("enct", [1024, 2048], BF, isOutput=False)
    encse_p = nc.declare_dram_parameter("encse", [64, 32 * 128], BF, isOutput=False)
    wordt_p = nc.declare_dram_parameter("wordt", [128, t_steps * 128], BF,
                                        isOutput=False)
    h0t_p = nc.declare_dram_parameter("h0t", [128, 8 * 32], BF, isOutput=False)
    c0s_p = nc.declare_dram_parameter("c0s", [32, 128], FP, isOutput=False)
    # scores stored transposed: [vocab_shard, t*32+b]
    out_p = nc.declare_dram_parameter("out", [VSH, CW], BF, isOutput=True)

    with tile.TileContext(nc) as tc:
        with (
            tc.tile_pool(name="res", bufs=1) as res,
            tc.tile_pool(name="wk", bufs=2) as wk,
            tc.tile_pool(name="ps1", bufs=1, space="PSUM") as ps1,
            tc.tile_pool(name="ps2", bufs=2, space="PSUM") as ps2,
            tc.tile_pool(name="dr", bufs=2, space="DRAM") as dr,
        ):
            # ---- resident SBUF ----
            w0s = res.tile([128, NK0 * 512], BF, tag="w0s")
            w1s = res.tile([128, NK1 * 512], BF, tag="w1s")
            wcs = res.tile([128, 16 * 128], BF, tag="wcs")
            at = res.tile([128, 2048], BF, tag="at")
            encse = res.tile([64, 32 * 128], BF, tag="encse")
            avhist = res.tile([128, 8 * CW], BF, tag="avhist")
            wotsb = res.tile([128, 8 * VSH], BF, tag="wotsb")
            wordsb = res.tile([128, t_steps * 128], BF, tag="wordsb")
            h1full = res.tile([128, 8 * 32], BF, tag="h1full")
            h2full = res.tile([128, 8 * 32], BF, tag="h2full")
            c = res.tile([32, 128], FP, tag="c")
            ones = res.tile([128, 32], BF, tag="ones")
            id32 = res.tile([32, 32], FP, tag="id32")
            id64 = res.tile([64, 64], FP, tag="id64")
            wpt = res.tile([128, 8 * 128], BF, tag="wpt")

            # ---- init loads (split for overlap) ----
            for kk in range(NK0):
                nc.sync.dma_start(out=w0s[:, 512 * kk:512 * (kk + 1)],
                                  in_=w0s_p[:, 512 * kk:512 * (kk + 1)])
            for kk in range(NK1):
                nc.sync.dma_start(out=w1s[:, 512 * kk:512 * (kk + 1)],
                                  in_=w1s_p[:, 512 * kk:512 * (kk + 1)])
            nc.sync.dma_start(out=wcs[:], in_=wcs_p[:])
            nc.sync.dma_start(out=encse[:], in_=encse_p[:])
            nc.sync.dma_start(out=h2full[:], in_=h0t_p[:])
            nc.sync.dma_start(out=c[:], in_=c0s_p[:])
            nc.sync.dma_start(out=wpt[:], in_=wpt_p[:])
            nc.sync.dma_start(out=wordsb[:], in_=wordt_p[:])
            for j in range(8):
                nc.scalar.dma_start(
                    out=wotsb[:, VSH * j:VSH * (j + 1)],
                    in_=wot_p[128 * j:128 * (j + 1), :])

            nc.vector.memset(ones[:], 0.0)
            nc.vector.memset(ones[0:1, :], 1.0)
            make_identity(nc, id32[:])
            make_identity(nc, id64[:])

            # ---- attention scores AT_shard = Wp_shard @ encT ----
            at_ps = [
                ps2.tile([128, 512], FP, tag="mm", name="atps_0"),
                ps2.tile([128, 512], FP, tag="mm", name="atps_1"),
                ps2.tile([128, 512], FP, tag="tr", name="atps_2"),
                ps1.tile([128, 512], FP, tag="av", name="atps_3"),
            ]
            for kk in range(8):
                et = wk.tile([128, 2048], BF, tag="enct", bufs=1)
                nc.sync.dma_start(out=et[:], in_=enct_p[128 * kk:128 * (kk + 1), :])
                for nch in range(4):
                    nc.tensor.matmul(at_ps[nch][:],
                                     wpt[:, 128 * kk:128 * (kk + 1)],
                                     et[:, 512 * nch:512 * (nch + 1)],
                                     start=(kk == 0), stop=(kk == 7))
            for nch in range(4):
                nc.scalar.activation(at[:, 512 * nch:512 * (nch + 1)],
                                     at_ps[nch][:], AF.Copy)

            # output projection (transposed): outT[v, (t,b)] += WoT.T @ av
            # chunk k: n8 = k // 32 covers cols [PW*n8, PW*n8+PW) = steps
            # [8*n8, 8*n8+8); vt = k % 32 covers vocab rows [128*vt, ...).
            NCHUNK = (CW // PW) * 32
            kctr = [0]

            def _emit_pb(k):
                n8, vt = k // 32, k % 32
                base = PW * n8
                mv = min(128, VSH - 128 * vt)
                bp = ps2.tile([mv, PW], FP, tag="g0", name=f"pb_{k}")
                for j in range(8):
                    nc.tensor.matmul(
                        bp[:], wotsb[:, VSH * j + 128 * vt:VSH * j + 128 * vt + mv],
                        avhist[:, j * CW + base:j * CW + base + PW],
                        start=(j == 0), stop=(j == 7))
                bs_ = wk.tile([mv, PW], BF, tag="bstg", name=f"pbs_{k}")
                nc.vector.tensor_copy(bs_[:], bp[:])
                nc.scalar.dma_start(
                    out=out_p[128 * vt:128 * vt + mv, base:base + PW],
                    in_=bs_[:])

            def _proj_slot(t, max_chunks=2):
                done = 0
                while (done < max_chunks and kctr[0] < NCHUNK
                       and 8 * (kctr[0] // 32) + 8 <= t):
                    _emit_pb(kctr[0])
                    kctr[0] += 1
                    done += 1

            # ---- recurrence ----
            for t in range(t_steps):
                # gates0 part 1: h2 / word / bias contributions (fills the
                # AR(t-1) flight on the PE; h2full(t-1) is loaded right
                # after AG(bm, t-1) completes)
                g0 = ps2.tile([32, 512], FP, tag="g0")
                mms = []
                for j in range(8):
                    mms.append((h2full[:, 32 * j:32 * (j + 1)], j))
                for j in range(4):
                    mms.append((wordsb[:, 128 * t + 32 * j:128 * t + 32 * (j + 1)],
                                8 + j))
                mms.append((ones[:], 12))
                for i, (lhsT, kk) in enumerate(mms):
                    nc.tensor.matmul(g0[:], lhsT,
                                     w0s[:, 512 * kk:512 * (kk + 1)],
                                     start=(i == 0),
                                     stop=(t == 0 and i == len(mms) - 1))

                # av(t-1): load summed avpre from AR output, tanh into
                # avhist column t-1, then accumulate av part of gates0
                if t > 0:
                    avpre = wk.tile([128, 256], BF, tag="avpre")
                    nc.sync.dma_start(out=avpre[:], in_=oav_prev[:])
                    dst = (avhist[:].rearrange("p (j c) -> p j c", c=CW)
                           [:, :, 32 * (t - 1):32 * t])
                    nc.scalar.activation(
                        dst, avpre[:].rearrange("p (j b) -> p j b", b=32),
                        AF.Tanh)
                    for j in range(8):
                        nc.tensor.matmul(
                            g0[:],
                            avhist[:, j * CW + 32 * (t - 1):j * CW + 32 * t],
                            w0s[:, 512 * (13 + j):512 * (14 + j)],
                            start=False, stop=(j == 7))
                # t==0: av contribution omitted (prev context is zeros in
                # the reference init); accumulation closed in part 1.

                # lstm cell 0 (gate order i,f,o,g along free dim)
                sifo = wk.tile([32, 384], FP, tag="sifo")
                tg = wk.tile([32, 128], FP, tag="tg")
                nc.scalar.activation(sifo[:], g0[:, 0:384], AF.Sigmoid)
                nc.scalar.activation(tg[:], g0[:, 384:512], AF.Tanh)
                t1 = wk.tile([32, 128], FP, tag="t1")
                t2 = wk.tile([32, 128], FP, tag="t2")
                nc.vector.tensor_mul(t1[:], sifo[:, 128:256], c[:])
                nc.vector.tensor_mul(t2[:], sifo[:, 0:128], tg[:])
                nc.vector.tensor_add(c[:], t1[:], t2[:])
                tc1 = wk.tile([32, 128], FP, tag="tc1")
                nc.scalar.activation(tc1[:], c[:], AF.Tanh)
                h1 = wk.tile([32, 128], FP, tag="h1")
                nc.vector.tensor_mul(h1[:], sifo[:, 256:384], tc1[:])

                # h1 -> h1T shard, AllGather -> h1full
                trp = ps2.tile([128, 32], FP, tag="tr")
                nc.tensor.transpose(trp[:], h1[:], id32[:])
                h1t = wk.tile([128, 32], BF, tag="h1t")
                nc.scalar.activation(h1t[:], trp[:], AF.Copy)
                b1 = dr.tile([128, 32], BF, tag="b1")
                o1 = dr.tile([1024, 32], BF, tag="o1")
                nc.sync.dma_start(out=b1[:], in_=h1t[:])
                nc.gpsimd.collective_compute(
                    "AllGather", mybir.AluOpType.bypass,
                    replica_groups=RG, ins=[b1.opt()], outs=[o1.opt()])

                # PE filler during AG(h1) flight
                _proj_slot(t)

                nc.sync.dma_start(
                    out=h1full[:].rearrange("p (j b) -> p j b", b=32),
                    in_=o1[:].rearrange("(j p) b -> p j b", p=128))

                # gates1: g1 = W1 @ [h1; 1]
                g1 = ps2.tile([32, 512], FP, tag="mm")
                for j in range(8):
                    nc.tensor.matmul(g1[:], h1full[:, 32 * j:32 * (j + 1)],
                                     w1s[:, 512 * j:512 * (j + 1)],
                                     start=(j == 0), stop=False)
                nc.tensor.matmul(g1[:], ones[:], w1s[:, 512 * 8:512 * 9],
                                 start=False, stop=True)

                # lstm cell 1 (same c state threaded; gate order i,f,o,g)
                sifo2 = wk.tile([32, 384], FP, tag="sifo")
                tg2 = wk.tile([32, 128], FP, tag="tg")
                nc.scalar.activation(sifo2[:], g1[:, 0:384], AF.Sigmoid)
                nc.scalar.activation(tg2[:], g1[:, 384:512], AF.Tanh)
                nc.vector.tensor_mul(t1[:], sifo2[:, 128:256], c[:])
                nc.vector.tensor_mul(t2[:], sifo2[:, 0:128], tg2[:])
                nc.vector.tensor_add(c[:], t1[:], t2[:])
                nc.scalar.activation(tc1[:], c[:], AF.Tanh)
                h2 = wk.tile([32, 128], FP, tag="h2")
                nc.vector.tensor_mul(h2[:], sifo2[:, 256:384], tc1[:])

                # h2 -> h2T shard
                trp2 = ps2.tile([128, 32], FP, tag="tr")
                nc.tensor.transpose(trp2[:], h2[:], id32[:])
                h2t = wk.tile([128, 32], BF, tag="h2t")
                nc.scalar.activation(h2t[:], trp2[:], AF.Copy)

                # logits partial per-batch ([s, b]), then transpose to
                # [b, s] so the bm staging write is 32 contiguous 128B
                # segments (sub-32B scattered HBM writes are catastrophic)
                lgps = ps2.tile([64, 32], FP, tag="tr", name="lgps")
                for b in range(32):
                    nc.tensor.matmul(lgps[:, b:b + 1],
                                     at[:, 64 * b:64 * (b + 1)],
                                     h2t[:, b:b + 1],
                                     start=True, stop=True)
                lgf = wk.tile([64, 32], FP, tag="lgf")
                nc.vector.tensor_copy(lgf[:], lgps[:])
                lgt_ps = ps2.tile([32, 64], FP, tag="tr", name="lgt_ps")
                nc.tensor.transpose(lgt_ps[:], lgf[:], id64[:])
                lgc = wk.tile([32, 64], BF, tag="lgc")
                nc.vector.tensor_copy(lgc[:], lgt_ps[:])

                # one merged AllGather: [h2T shard (128x32) | logits (b-major)]
                bm = dr.tile([192, 32], BF, tag="bm")
                om = dr.tile([1536, 32], BF, tag="om")
                nc.sync.dma_start(out=bm[0:128, :], in_=h2t[:])
                bm_ap = bm[:]
                # flat bm index for logit (b, s) is 128*32 + 64b + s; lgc is
                # [b, s] so this is 32 contiguous 128B rows
                bm_lp = bass_rust.AP(bm_ap.tensor, bm_ap.offset + 128 * 32,
                                     [[64, 32], [1, 64]])
                nc.sync.dma_start(out=bm_lp, in_=lgc[:])
                nc.gpsimd.collective_compute(
                    "AllGather", mybir.AluOpType.bypass,
                    replica_groups=RG, ins=[bm.opt()], outs=[om.opt()])

                # PE filler during AG(bm) flight
                _proj_slot(t)

                om_ap = om[:]
                # h2full[p, j*32+b] = om[j*192 + p, b]
                nc.sync.dma_start(
                    out=h2full[:].rearrange("p (j b) -> p j b", b=32),
                    in_=bass_rust.AP(om_ap.tensor, om_ap.offset,
                                     [[32, 128], [192 * 32, 8], [1, 32]]))
                # ls[b, r*64+s] = om[r*192 + 128 + (64b+s)//32, (64b+s)%32]
                ls = wk.tile([32, 8 * 64], BF, tag="ls")
                nc.sync.dma_start(
                    out=ls[:].rearrange("p (r s) -> p r s", s=64),
                    in_=bass_rust.AP(om_ap.tensor, om_ap.offset + 128 * 32,
                                     [[64, 32], [192 * 32, 8], [1, 64]]))
                # tree sum of the 8 rank partials
                lv1 = wk.tile([32, 256], FP, tag="lv1")
                lv2 = wk.tile([32, 128], FP, tag="lv2")
                lg0 = wk.tile([32, 64], FP, tag="lgs")
                nc.vector.tensor_add(lv1[:], ls[:, 0:256], ls[:, 256:512])
                nc.vector.tensor_add(lv2[:], lv1[:, 0:128], lv1[:, 128:256])
                nc.vector.tensor_add(lg0[:], lv2[:, 0:64], lv2[:, 64:128])

                # softmax over s
                mx = wk.tile([32, 1], FP, tag="mx")
                nc.vector.reduce_max(mx[:], lg0[:], axis=X, negate=True)
                ex = wk.tile([32, 64], FP, tag="ex")
                nc.scalar.activation(ex[:], lg0[:], AF.Exp, bias=mx[:])
                sm = wk.tile([32, 1], FP, tag="sm")
                nc.vector.reduce_sum(sm[:], ex[:], axis=X)
                rc = wk.tile([32, 1], FP, tag="rc")
                nc.vector.reciprocal(rc[:], sm[:])
                al = wk.tile([32, 64], FP, tag="al")
                nc.vector.tensor_scalar_mul(al[:], ex[:], rc[:])

                # alpha -> alphaT
                trp3 = ps2.tile([64, 32], FP, tag="tr")
                nc.tensor.transpose(trp3[:], al[:], id32[:])
                alt = wk.tile([64, 32], BF, tag="alt")
                nc.scalar.activation(alt[:], trp3[:], AF.Copy)

                # context per-batch (transposed): cxt[:, b] = encse_b.T @ alpha_b
                cxt_ps = ps2.tile([128, 32], FP, tag="tr")
                for b in range(32):
                    nc.tensor.matmul(cxt_ps[:, b:b + 1],
                                     encse[:, 128 * b:128 * (b + 1)],
                                     alt[:, b:b + 1],
                                     start=True, stop=True)
                cxt = wk.tile([128, 32], BF, tag="cxt")
                nc.scalar.activation(cxt[:], cxt_ps[:], AF.Copy)

                # combine partial: av_preT[m-dims, b] over own 256 K dims
                avp = ps1.tile([128, 256], FP, tag="av")
                for m in range(8):
                    nc.tensor.matmul(avp[:, 32 * m:32 * (m + 1)],
                                     wcs[:, (0 * 8 + m) * 128:(0 * 8 + m) * 128 + 128],
                                     h2t[:], start=True, stop=False)
                    nc.tensor.matmul(avp[:, 32 * m:32 * (m + 1)],
                                     wcs[:, (1 * 8 + m) * 128:(1 * 8 + m) * 128 + 128],
                                     cxt[:], start=False, stop=True)
                avs = wk.tile([128, 256], BF, tag="avs")
                nc.vector.tensor_copy(avs[:], avp[:])
                # AllReduce in the native [128, 256] layout: staging and
                # readback are fully contiguous per partition (512B segs)
                bav = dr.tile([128, 256], BF, tag="bav")
                oav = dr.tile([128, 256], BF, tag="oav")
                nc.sync.dma_start(out=bav[:], in_=avs[:])
                nc.gpsimd.collective_compute(
                    "AllReduce", mybir.AluOpType.add,
                    replica_groups=RG, ins=[bav.opt()], outs=[oav.opt()])
                oav_prev = oav

                # PE filler during AR flight
                _proj_slot(t)

            # ---- final av column + remaining projection chunks ----
            avpre = wk.tile([128, 256], BF, tag="avpre")
            nc.sync.dma_start(out=avpre[:], in_=oav_prev[:])
            dst = (avhist[:].rearrange("p (j c) -> p j c", c=CW)
                   [:, :, 32 * (t_steps - 1):32 * t_steps])
            nc.scalar.activation(
                dst, avpre[:].rearrange("p (j b) -> p j b", b=32), AF.Tanh)
            while kctr[0] < NCHUNK:
                _emit_pb(kctr[0])
                kctr[0] += 1

    nc.compile()
    return nc


def _prep(inputs, t_steps=T):
    g = {k: np.asarray(v) for k, v in inputs.items()}
    src = g["src_encodings"].astype(np.float32)          # [S, B, 2E]
    h0 = g["h0"].astype(np.float32)
    c0 = g["c0"].astype(np.float32)
    emb = g["embedding"].astype(np.float32)
    Wp = g["W_proj"].astype(np.float32)
    Wc = g["W_combine"].astype(np.float32)
    Wo = g["W_out"].astype(np.float32)
    Wih0 = g["W_ih0"].astype(np.float32)
    Whh0 = g["W_hh0"].astype(np.float32)
    bih0 = g["b_ih0"].astype(np.float32)
    bhh0 = g["b_hh0"].astype(np.float32)
    Wih1 = g["W_ih1"].astype(np.float32)
    Whh1 = g["W_hh1"].astype(np.float32)
    bih1 = g["b_ih1"].astype(np.float32)
    bhh1 = g["b_hh1"].astype(np.float32)
    tgt = np.asarray(g["tgt_tensor"]).astype(np.int64)   # [T, B]

    W1 = Wih1 + Whh1
    b0 = bih0 + bhh0
    b1 = bih1 + bhh1

    # shared across cores
    wemb = emb[tgt[:t_steps]]                            # [t, B, E]
    # wordt: [128, t*128]; step block t = wordT[:,t] split into 4 j-blocks
    wordt = (wemb.transpose(0, 2, 1)                     # [t, E, B]
             .reshape(t_steps, 4, 128, 32)
             .transpose(2, 0, 1, 3).reshape(128, t_steps * 128))
    wordt = np.ascontiguousarray(wordt).astype(NBF)
    enct = np.ascontiguousarray(
        src.transpose(2, 1, 0).reshape(1024, 2048)).astype(NBF)  # [e, b*64+s]
    h0t = np.ascontiguousarray(
        h0.T.reshape(8, 128, 32).transpose(1, 0, 2).reshape(128, 256)).astype(NBF)

    in_maps = []
    for k in range(P):
        rows = np.concatenate([gg * 1024 + k * 128 + np.arange(128)
                               for gg in (0, 1, 3, 2)])  # [i|f|o|g] x 128 dims
        # W0sT_aug rows: [h2 1024 | word 512 | bias 1 | pad | av 1024]
        w0a = np.zeros((NK0 * 128, 512), np.float32)
        w0a[0:1024] = Whh0[rows].T
        w0a[1024:1536] = Wih0[rows, 0:512].T
        w0a[1536] = b0[rows]
        w0a[1664:2688] = Wih0[rows, 512:1536].T
        w0s = np.ascontiguousarray(
            w0a.reshape(NK0, 128, 512).transpose(1, 0, 2)
            .reshape(128, NK0 * 512)).astype(NBF)

        w1a = np.zeros((NK1 * 128, 512), np.float32)
        w1a[0:1024] = W1[rows].T
        w1a[1024] = b1[rows]
        w1s = np.ascontiguousarray(
            w1a.reshape(NK1, 128, 512).transpose(1, 0, 2)
            .reshape(128, NK1 * 512)).astype(NBF)

        # Wc own-K slice: h dims [128k..] and ctx dims [1024+128k..]
        hs = slice(k * 128, k * 128 + 128)
        cs = slice(1024 + k * 128, 1024 + k * 128 + 128)
        wc_own = np.concatenate([Wc[:, hs], Wc[:, cs]], axis=1)  # [1024, 256]
        blocks = []
        for j in range(2):
            for m in range(8):
                blocks.append(wc_own[128 * m:128 * (m + 1),
                                     128 * j:128 * (j + 1)].T)
        wcs = np.ascontiguousarray(np.concatenate(blocks, axis=1)).astype(NBF)

        wot = np.ascontiguousarray(Wo[VSH * k:VSH * (k + 1)].T).astype(NBF)
        wpt_ = Wp[128 * k:128 * (k + 1), :].T                       # [1024, 128]
        wpt = np.ascontiguousarray(
            wpt_.reshape(8, 128, 128).transpose(1, 0, 2)
            .reshape(128, 8 * 128)).astype(NBF)
        # encse2[s, b*128 + e] = src[s, b, e_shard]
        encse = np.ascontiguousarray(
            src[:, :, 128 * k:128 * (k + 1)].reshape(64, 32 * 128)).astype(NBF)
        c0s = np.ascontiguousarray(c0[:, 128 * k:128 * (k + 1)])

        in_maps.append({
            "w0s": w0s, "w1s": w1s, "wcs": wcs, "wot": wot, "wpt": wpt,
            "enct": enct, "encse": encse, "wordt": wordt,
            "h0t": h0t, "c0s": c0s,
        })
    return in_maps


_CACHE = {}


def _get_nc(t_steps=T):
    if t_steps not in _CACHE:
        _CACHE[t_steps] = _build(t_steps)
    return _CACHE[t_steps]


def run_device(inputs, trace=False, t_steps=T):
    nc = _get_nc(t_steps)
    in_maps = _prep(inputs, t_steps)
    return run_bass_kernel_spmd(nc, in_maps, core_ids=list(range(P)), trace=trace)


def assemble(results, t_steps=T):
    return np.concatenate(
        [np.ascontiguousarray(np.asarray(results[k]["out"]).astype(np.float32).T)
         .reshape(t_steps, B, VSH) for k in range(P)],
        axis=2)


def kernel(**inputs):
    r = run_device(inputs)
    return assemble(r.results)


# revision 15
# speedup vs baseline: 1.5618x; 1.0401x over previous
"""Trainium2 Bass kernel for nn_Decoder (LSTM decoder w/ attention).

Sharding: 8-way model parallel over hidden dim D for the recurrence
(each core owns 128 of 1024 dims = all 4 gates for those dims), vocab
shard (4000 rows/core) for the output projection, which runs as a
batched matmul over all T*B rows interleaved with the recurrence.

All matmul operands are bf16 (1 cycle/row on the PE vs 4 for fp32);
accumulation stays fp32 in PSUM, LSTM cell state and softmax stay fp32.
Collective payloads and the final score store are bf16 (host casts back).

Perf-critical details (learned from NTFF traces):
- Collective input stagings must be contiguous >=32B segments: sub-32B
  scattered HBM writes trigger read-modify-write and delay the DMA
  completion semaphore by ~20us, stalling the next collective trigger.
  Logits are therefore computed directly in [b, s] orientation and the
  AllReduce buffers keep the natural [128, 256] layout.
- Projection chunks (width 256) are interleaved into all three
  per-step collective flights to keep the PE busy (p-state) and to
  absorb the ~24us/step of collective latency.
- Projection stores issue from the Scalar HWDGE queue so they never
  head-of-line block the Sync queue carrying recurrence DMAs.
"""

import numpy as np
import ml_dtypes
import bass_rust
import concourse.bass as bass  # noqa: F401  (bass types used via bacc)
import concourse.tile as tile
from concourse import bacc, mybir
from concourse.bass_utils import run_bass_kernel_spmd
from concourse.masks import make_identity

V, E, D = 32000, 512, 1024
TWO_E = 1024
B, S, T = 32, 64, 48
P = 8
DSH = D // P        # 128 hidden dims per core
VSH = V // P        # 4000 vocab rows per core
FP = mybir.dt.float32
BF = mybir.dt.bfloat16
NBF = ml_dtypes.bfloat16
AF = mybir.ActivationFunctionType
RG = [list(range(P))]
X = mybir.AxisListType.X

# gates0 lhsT layout: [h2 (8x128) | word (4x128) | ones/bias (128) | av (8x128)]
NK0 = 21
# gates1 lhsT layout: [h1 (8x128) | ones/bias (128)]
NK1 = 9

PW = 256            # projection chunk width (cols = t*32+b)


def _build(t_steps=T):
    nc = bacc.Bacc("TRN2", target_bir_lowering=False, debug=False, num_devices=P)
    CW = t_steps * 32  # avhist block width (cols = t*32+b)

    w0s_p = nc.declare_dram_parameter("w0s", [128, NK0 * 512], BF, isOutput=False)
    w1s_p = nc.declare_dram_parameter("w1s", [128, NK1 * 512], BF, isOutput=False)
    wcs_p = nc.declare_dram_parameter("wcs", [128, 16 * 128], BF, isOutput=False)
    wot_p = nc.declare_dram_parameter("wot", [1024, VSH], BF, isOutput=False)
    wpt_p = nc.declare_dram_parameter("wpt", [128, 8 * 128], BF, isOutput=False)
    enct_p = nc.declare_dram_parameter("enct", [1024, 2048], BF, isOutput=False)
    encse_p = nc.declare_dram_parameter("encse", [64, 32 * 128], BF, isOutput=False)
    wordt_p = nc.declare_dram_parameter("wordt", [128, t_steps * 128], BF,
                                        isOutput=False)
    h0t_p = nc.declare_dram_parameter("h0t", [128, 8 * 32], BF, isOutput=False)
    c0s_p = nc.declare_dram_parameter("c0s", [32, 128], FP, isOutput=False)
    # scores stored transposed: [vocab_shard, t*32+b]
    out_p = nc.declare_dram_parameter("out", [VSH, CW], BF, isOutput=True)

    with tile.TileContext(nc) as tc:
        with (
            tc.tile_pool(name="res", bufs=1) as res,
            tc.tile_pool(name="wk", bufs=2) as wk,
            tc.tile_pool(name="ps1", bufs=1, space="PSUM") as ps1,
            tc.tile_pool(name="ps2", bufs=2, space="PSUM") as ps2,
            tc.tile_pool(name="dr", bufs=2, space="DRAM") as dr,
        ):
            # ---- resident SBUF ----
            w0s = res.tile([128, NK0 * 512], BF, tag="w0s")
            w1s = res.tile([128, NK1 * 512], BF, tag="w1s")
            wcs = res.tile([128, 16 * 128], BF, tag="wcs")
            at = res.tile([128, 2048], BF, tag="at")
            encse = res.tile([64, 32 * 128], BF, tag="encse")
            avhist = res.tile([128, 8 * CW], BF, tag="avhist")
            wotsb = res.tile([128, 8 * VSH], BF, tag="wotsb")
            wordsb = res.tile([128, t_steps * 128], BF, tag="wordsb")
            h1full = res.tile([128, 8 * 32], BF, tag="h1full")
            h2full = res.tile([128, 8 * 32], BF, tag="h2full")
            c = res.tile([32, 128], FP, tag="c")
            ones = res.tile([128, 32], BF, tag="ones")
            id32 = res.tile([32, 32], FP, tag="id32")
            id64 = res.tile([64, 64], FP, tag="id64")
            wpt = res.tile([128, 8 * 128], BF, tag="wpt")

            # ---- init loads (split for overlap) ----
            for kk in range(NK0):
                nc.sync.dma_start(out=w0s[:, 512 * kk:512 * (kk + 1)],
                                  in_=w0s_p[:, 512 * kk:512 * (kk + 1)])
            for kk in range(NK1):
                nc.sync.dma_start(out=w1s[:, 512 * kk:512 * (kk + 1)],
                                  in_=w1s_p[:, 512 * kk:512 * (kk + 1)])
            nc.sync.dma_start(out=wcs[:], in_=wcs_p[:])
            nc.sync.dma_start(out=encse[:], in_=encse_p[:])
            nc.sync.dma_start(out=h2full[:], in_=h0t_p[:])
            nc.sync.dma_start(out=c[:], in_=c0s_p[:])
            nc.sync.dma_start(out=wpt[:], in_=wpt_p[:])
            nc.sync.dma_start(out=wordsb[:], in_=wordt_p[:])
            for j in range(8):
                nc.scalar.dma_start(
                    out=wotsb[:, VSH * j:VSH * (j + 1)],
                    in_=wot_p[128 * j:128 * (j + 1), :])

            nc.vector.memset(ones[:], 0.0)
            nc.vector.memset(ones[0:1, :], 1.0)
            make_identity(nc, id32[:])
            make_identity(nc, id64[:])

            # ---- attention scores AT_shard = Wp_shard @ encT ----
            at_ps = [
                ps2.tile([128, 512], FP, tag="mm", name="atps_0"),
                ps2.tile([128, 512], FP, tag="mm", name="atps_1"),
                ps2.tile([128, 512], FP, tag="tr", name="atps_2"),
                ps1.tile([128, 512], FP, tag="av", name="atps_3"),
            ]
            for kk in range(8):
                et = wk.tile([128, 2048], BF, tag="enct", bufs=1)
                nc.sync.dma_start(out=et[:], in_=enct_p[128 * kk:128 * (kk + 1), :])
                for nch in range(4):
                    nc.tensor.matmul(at_ps[nch][:],
                                     wpt[:, 128 * kk:128 * (kk + 1)],
                                     et[:, 512 * nch:512 * (nch + 1)],
                                     start=(kk == 0), stop=(kk == 7))
            for nch in range(4):
                nc.scalar.activation(at[:, 512 * nch:512 * (nch + 1)],
                                     at_ps[nch][:], AF.Copy)

            # output projection (transposed): outT[v, (t,b)] += WoT.T @ av
            # chunk k: n8 = k // 32 covers cols [PW*n8, PW*n8+PW) = steps
            # [8*n8, 8*n8+8); vt = k % 32 covers vocab rows [128*vt, ...).
            NCHUNK = (CW // PW) * 32
            kctr = [0]
            combine_last = [None]

            def _emit_pb(k):
                n8, vt = k // 32, k % 32
                base = PW * n8
                mv = min(128, VSH - 128 * vt)
                bp = ps2.tile([mv, PW], FP, tag="g0", name=f"pb_{k}")
                for j in range(8):
                    nc.tensor.matmul(
                        bp[:], wotsb[:, VSH * j + 128 * vt:VSH * j + 128 * vt + mv],
                        avhist[:, j * CW + base:j * CW + base + PW],
                        start=(j == 0), stop=(j == 7))
                bs_ = wk.tile([mv, PW], BF, tag="bstg", name=f"pbs_{k}")
                nc.vector.tensor_copy(bs_[:], bp[:])
                nc.scalar.dma_start(
                    out=out_p[128 * vt:128 * vt + mv, base:base + PW],
                    in_=bs_[:])

            def _proj_slot(t, max_chunks=2):
                done = 0
                while (done < max_chunks and kctr[0] < NCHUNK
                       and 8 * (kctr[0] // 32) + 8 <= t):
                    _emit_pb(kctr[0])
                    kctr[0] += 1
                    done += 1

            # ---- recurrence ----
            for t in range(t_steps):
                # gates0 part 1: h2 / word / bias contributions (fills the
                # AR(t-1) flight on the PE; h2full(t-1) is loaded right
                # after AG(bm, t-1) completes)
                g0 = ps2.tile([32, 512], FP, tag="g0")
                mms = []
                for j in range(8):
                    mms.append((h2full[:, 32 * j:32 * (j + 1)], j))
                for j in range(4):
                    mms.append((wordsb[:, 128 * t + 32 * j:128 * t + 32 * (j + 1)],
                                8 + j))
                mms.append((ones[:], 12))
                for i, (lhsT, kk) in enumerate(mms):
                    mm = nc.tensor.matmul(g0[:], lhsT,
                                          w0s[:, 512 * kk:512 * (kk + 1)],
                                          start=(i == 0),
                                          stop=(t == 0 and i == len(mms) - 1))
                    if i == 0 and combine_last[0] is not None:
                        # keep the PE from hoisting next-step gate matmuls
                        # ahead of the time-critical softmax->AR path
                        bass_rust.add_dep_helper(
                            mm.ins, combine_last[0].ins, sync=False,
                            reason="g0 part1 after combine")

                # av(t-1): load summed avpre from AR output, tanh into
                # avhist column t-1, then accumulate av part of gates0
                if t > 0:
                    avpre = wk.tile([128, 256], BF, tag="avpre")
                    nc.sync.dma_start(out=avpre[:], in_=oav_prev[:])
                    dst = (avhist[:].rearrange("p (j c) -> p j c", c=CW)
                           [:, :, 32 * (t - 1):32 * t])
                    nc.scalar.activation(
                        dst, avpre[:].rearrange("p (j b) -> p j b", b=32),
                        AF.Tanh)
                    for j in range(8):
                        nc.tensor.matmul(
                            g0[:],
                            avhist[:, j * CW + 32 * (t - 1):j * CW + 32 * t],
                            w0s[:, 512 * (13 + j):512 * (14 + j)],
                            start=False, stop=(j == 7))
                # t==0: av contribution omitted (prev context is zeros in
                # the reference init); accumulation closed in part 1.

                # lstm cell 0 (gate order i,f,o,g along free dim)
                sifo = wk.tile([32, 384], FP, tag="sifo")
                tg = wk.tile([32, 128], FP, tag="tg")
                nc.scalar.activation(sifo[:], g0[:, 0:384], AF.Sigmoid)
                nc.scalar.activation(tg[:], g0[:, 384:512], AF.Tanh)
                t1 = wk.tile([32, 128], FP, tag="t1")
                t2 = wk.tile([32, 128], FP, tag="t2")
                nc.vector.tensor_mul(t1[:], sifo[:, 128:256], c[:])
                nc.vector.tensor_mul(t2[:], sifo[:, 0:128], tg[:])
                nc.vector.tensor_add(c[:], t1[:], t2[:])
                tc1 = wk.tile([32, 128], FP, tag="tc1")
                nc.scalar.activation(tc1[:], c[:], AF.Tanh)
                h1 = wk.tile([32, 128], FP, tag="h1")
                nc.vector.tensor_mul(h1[:], sifo[:, 256:384], tc1[:])

                # h1 -> h1T shard, AllGather -> h1full
                trp = ps2.tile([128, 32], FP, tag="tr")
                nc.tensor.transpose(trp[:], h1[:], id32[:])
                h1t = wk.tile([128, 32], BF, tag="h1t")
                nc.scalar.activation(h1t[:], trp[:], AF.Copy)
                b1 = dr.tile([128, 32], BF, tag="b1")
                o1 = dr.tile([1024, 32], BF, tag="o1", addr_space="Shared")
                nc.sync.dma_start(out=b1[:], in_=h1t[:])
                nc.gpsimd.collective_compute(
                    "AllGather", mybir.AluOpType.bypass,
                    replica_groups=RG, ins=[b1.opt()], outs=[o1.opt()])

                # PE filler during AG(h1) flight
                _proj_slot(t)

                nc.sync.dma_start(
                    out=h1full[:].rearrange("p (j b) -> p j b", b=32),
                    in_=o1[:].rearrange("(j p) b -> p j b", p=128))

                # gates1: g1 = W1 @ [h1; 1]
                g1 = ps2.tile([32, 512], FP, tag="mm")
                for j in range(8):
                    nc.tensor.matmul(g1[:], h1full[:, 32 * j:32 * (j + 1)],
                                     w1s[:, 512 * j:512 * (j + 1)],
                                     start=(j == 0), stop=False)
                nc.tensor.matmul(g1[:], ones[:], w1s[:, 512 * 8:512 * 9],
                                 start=False, stop=True)

                # lstm cell 1 (same c state threaded; gate order i,f,o,g)
                sifo2 = wk.tile([32, 384], FP, tag="sifo")
                tg2 = wk.tile([32, 128], FP, tag="tg")
                nc.scalar.activation(sifo2[:], g1[:, 0:384], AF.Sigmoid)
                nc.scalar.activation(tg2[:], g1[:, 384:512], AF.Tanh)
                nc.vector.tensor_mul(t1[:], sifo2[:, 128:256], c[:])
                nc.vector.tensor_mul(t2[:], sifo2[:, 0:128], tg2[:])
                nc.vector.tensor_add(c[:], t1[:], t2[:])
                nc.scalar.activation(tc1[:], c[:], AF.Tanh)
                h2 = wk.tile([32, 128], FP, tag="h2")
                nc.vector.tensor_mul(h2[:], sifo2[:, 256:384], tc1[:])

                # h2 -> h2T shard
                trp2 = ps2.tile([128, 32], FP, tag="tr")
                nc.tensor.transpose(trp2[:], h2[:], id32[:])
                h2t = wk.tile([128, 32], BF, tag="h2t")
                nc.scalar.activation(h2t[:], trp2[:], AF.Copy)

                # logits partial per-batch ([s, b]), then transpose to
                # [b, s] so the bm staging write is 32 contiguous 128B
                # segments (sub-32B scattered HBM writes are catastrophic)
                lgps = ps2.tile([64, 32], FP, tag="tr", name="lgps")
                for b in range(32):
                    nc.tensor.matmul(lgps[:, b:b + 1],
                                     at[:, 64 * b:64 * (b + 1)],
                                     h2t[:, b:b + 1],
                                     start=True, stop=True)
                lgf = wk.tile([64, 32], FP, tag="lgf")
                nc.vector.tensor_copy(lgf[:], lgps[:])
                lgt_ps = ps2.tile([32, 64], FP, tag="tr", name="lgt_ps")
                nc.tensor.transpose(lgt_ps[:], lgf[:], id64[:])
                lgc = wk.tile([32, 64], BF, tag="lgc")
                nc.vector.tensor_copy(lgc[:], lgt_ps[:])

                # one merged AllGather: [h2T shard (128x32) | logits (b-major)]
                bm = dr.tile([192, 32], BF, tag="bm")
                om = dr.tile([1536, 32], BF, tag="om", addr_space="Shared")
                nc.sync.dma_start(out=bm[0:128, :], in_=h2t[:])
                bm_ap = bm[:]
                # flat bm index for logit (b, s) is 128*32 + 64b + s; lgc is
                # [b, s] so this is 32 contiguous 128B rows
                bm_lp = bass_rust.AP(bm_ap.tensor, bm_ap.offset + 128 * 32,
                                     [[64, 32], [1, 64]])
                nc.sync.dma_start(out=bm_lp, in_=lgc[:])
                nc.gpsimd.collective_compute(
                    "AllGather", mybir.AluOpType.bypass,
                    replica_groups=RG, ins=[bm.opt()], outs=[om.opt()])

                # PE filler during AG(bm) flight
                _proj_slot(t)

                om_ap = om[:]
                # h2full[p, j*32+b] = om[j*192 + p, b]
                nc.sync.dma_start(
                    out=h2full[:].rearrange("p (j b) -> p j b", b=32),
                    in_=bass_rust.AP(om_ap.tensor, om_ap.offset,
                                     [[32, 128], [192 * 32, 8], [1, 32]]))
                # ls[b, r*64+s] = om[r*192 + 128 + (64b+s)//32, (64b+s)%32]
                ls = wk.tile([32, 8 * 64], BF, tag="ls")
                nc.sync.dma_start(
                    out=ls[:].rearrange("p (r s) -> p r s", s=64),
                    in_=bass_rust.AP(om_ap.tensor, om_ap.offset + 128 * 32,
                                     [[64, 32], [192 * 32, 8], [1, 64]]))
                # tree sum of the 8 rank partials
                lv1 = wk.tile([32, 256], FP, tag="lv1")
                lv2 = wk.tile([32, 128], FP, tag="lv2")
                lg0 = wk.tile([32, 64], FP, tag="lgs")
                nc.vector.tensor_add(lv1[:], ls[:, 0:256], ls[:, 256:512])
                nc.vector.tensor_add(lv2[:], lv1[:, 0:128], lv1[:, 128:256])
                nc.vector.tensor_add(lg0[:], lv2[:, 0:64], lv2[:, 64:128])

                # softmax over s
                mx = wk.tile([32, 1], FP, tag="mx")
                nc.vector.reduce_max(mx[:], lg0[:], axis=X, negate=True)
                ex = wk.tile([32, 64], FP, tag="ex")
                nc.scalar.activation(ex[:], lg0[:], AF.Exp, bias=mx[:])
                sm = wk.tile([32, 1], FP, tag="sm")
                nc.vector.reduce_sum(sm[:], ex[:], axis=X)
                rc = wk.tile([32, 1], FP, tag="rc")
                nc.vector.reciprocal(rc[:], sm[:])
                al = wk.tile([32, 64], FP, tag="al")
                nc.vector.tensor_scalar_mul(al[:], ex[:], rc[:])

                # alpha -> alphaT
                trp3 = ps2.tile([64, 32], FP, tag="tr")
                nc.tensor.transpose(trp3[:], al[:], id32[:])
                alt = wk.tile([64, 32], BF, tag="alt")
                nc.scalar.activation(alt[:], trp3[:], AF.Copy)

                # context per-batch (transposed): cxt[:, b] = encse_b.T @ alpha_b
                cxt_ps = ps2.tile([128, 32], FP, tag="tr")
                for b in range(32):
                    nc.tensor.matmul(cxt_ps[:, b:b + 1],
                                     encse[:, 128 * b:128 * (b + 1)],
                                     alt[:, b:b + 1],
                                     start=True, stop=True)
                cxt = wk.tile([128, 32], BF, tag="cxt")
                nc.scalar.activation(cxt[:], cxt_ps[:], AF.Copy)

                # combine partial: av_preT[m-dims, b] over own 256 K dims
                avp = ps1.tile([128, 256], FP, tag="av")
                for m in range(8):
                    nc.tensor.matmul(avp[:, 32 * m:32 * (m + 1)],
                                     wcs[:, (0 * 8 + m) * 128:(0 * 8 + m) * 128 + 128],
                                     h2t[:], start=True, stop=False)
                    combine_last[0] = nc.tensor.matmul(
                        avp[:, 32 * m:32 * (m + 1)],
                        wcs[:, (1 * 8 + m) * 128:(1 * 8 + m) * 128 + 128],
                        cxt[:], start=False, stop=True)
                avs = wk.tile([128, 256], BF, tag="avs")
                nc.vector.tensor_copy(avs[:], avp[:])
                # AllReduce in the native [128, 256] layout: staging and
                # readback are fully contiguous per partition (512B segs)
                bav = dr.tile([128, 256], BF, tag="bav")
                oav = dr.tile([128, 256], BF, tag="oav", addr_space="Shared")
                nc.sync.dma_start(out=bav[:], in_=avs[:])
                nc.gpsimd.collective_compute(
                    "AllReduce", mybir.AluOpType.add,
                    replica_groups=RG, ins=[bav.opt()], outs=[oav.opt()])
                oav_prev = oav

                # PE filler during AR flight
                _proj_slot(t)

            # ---- final av column + remaining projection chunks ----
            avpre = wk.tile([128, 256], BF, tag="avpre")
            nc.sync.dma_start(out=avpre[:], in_=oav_prev[:])
            dst = (avhist[:].rearrange("p (j c) -> p j c", c=CW)
                   [:, :, 32 * (t_steps - 1):32 * t_steps])
            nc.scalar.activation(
                dst, avpre[:].rearrange("p (j b) -> p j b", b=32), AF.Tanh)
            while kctr[0] < NCHUNK:
                _emit_pb(kctr[0])
                kctr[0] += 1

    nc.compile()
    return nc


def _prep(inputs, t_steps=T):
    g = {k: np.asarray(v) for k, v in inputs.items()}
    src = g["src_encodings"].astype(np.float32)          # [S, B, 2E]
    h0 = g["h0"].astype(np.float32)
    c0 = g["c0"].astype(np.float32)
    emb = g["embedding"].astype(np.float32)
    Wp = g["W_proj"].astype(np.float32)
    Wc = g["W_combine"].astype(np.float32)
    Wo = g["W_out"].astype(np.float32)
    Wih0 = g["W_ih0"].astype(np.float32)
    Whh0 = g["W_hh0"].astype(np.float32)
    bih0 = g["b_ih0"].astype(np.float32)
    bhh0 = g["b_hh0"].astype(np.float32)
    Wih1 = g["W_ih1"].astype(np.float32)
    Whh1 = g["W_hh1"].astype(np.float32)
    bih1 = g["b_ih1"].astype(np.float32)
    bhh1 = g["b_hh1"].astype(np.float32)
    tgt = np.asarray(g["tgt_tensor"]).astype(np.int64)   # [T, B]

    W1 = Wih1 + Whh1
    b0 = bih0 + bhh0
    b1 = bih1 + bhh1

    # shared across cores
    wemb = emb[tgt[:t_steps]]                            # [t, B, E]
    # wordt: [128, t*128]; step block t = wordT[:,t] split into 4 j-blocks
    wordt = (wemb.transpose(0, 2, 1)                     # [t, E, B]
             .reshape(t_steps, 4, 128, 32)
             .transpose(2, 0, 1, 3).reshape(128, t_steps * 128))
    wordt = np.ascontiguousarray(wordt).astype(NBF)
    enct = np.ascontiguousarray(
        src.transpose(2, 1, 0).reshape(1024, 2048)).astype(NBF)  # [e, b*64+s]
    h0t = np.ascontiguousarray(
        h0.T.reshape(8, 128, 32).transpose(1, 0, 2).reshape(128, 256)).astype(NBF)

    in_maps = []
    for k in range(P):
        rows = np.concatenate([gg * 1024 + k * 128 + np.arange(128)
                               for gg in (0, 1, 3, 2)])  # [i|f|o|g] x 128 dims
        # W0sT_aug rows: [h2 1024 | word 512 | bias 1 | pad | av 1024]
        w0a = np.zeros((NK0 * 128, 512), np.float32)
        w0a[0:1024] = Whh0[rows].T
        w0a[1024:1536] = Wih0[rows, 0:512].T
        w0a[1536] = b0[rows]
        w0a[1664:2688] = Wih0[rows, 512:1536].T
        w0s = np.ascontiguousarray(
            w0a.reshape(NK0, 128, 512).transpose(1, 0, 2)
            .reshape(128, NK0 * 512)).astype(NBF)

        w1a = np.zeros((NK1 * 128, 512), np.float32)
        w1a[0:1024] = W1[rows].T
        w1a[1024] = b1[rows]
        w1s = np.ascontiguousarray(
            w1a.reshape(NK1, 128, 512).transpose(1, 0, 2)
            .reshape(128, NK1 * 512)).astype(NBF)

        # Wc own-K slice: h dims [128k..] and ctx dims [1024+128k..]
        hs = slice(k * 128, k * 128 + 128)
        cs = slice(1024 + k * 128, 1024 + k * 128 + 128)
        wc_own = np.concatenate([Wc[:, hs], Wc[:, cs]], axis=1)  # [1024, 256]
        blocks = []
        for j in range(2):
            for m in range(8):
                blocks.append(wc_own[128 * m:128 * (m + 1),
                                     128 * j:128 * (j + 1)].T)
        wcs = np.ascontiguousarray(np.concatenate(blocks, axis=1)).astype(NBF)

        wot = np.ascontiguousarray(Wo[VSH * k:VSH * (k + 1)].T).astype(NBF)
        wpt_ = Wp[128 * k:128 * (k + 1), :].T                       # [1024, 128]
        wpt = np.ascontiguousarray(
            wpt_.reshape(8, 128, 128).transpose(1, 0, 2)
            .reshape(128, 8 * 128)).astype(NBF)
        # encse2[s, b*128 + e] = src[s, b, e_shard]
        encse = np.ascontiguousarray(
            src[:, :, 128 * k:128 * (k + 1)].reshape(64, 32 * 128)).astype(NBF)
        c0s = np.ascontiguousarray(c0[:, 128 * k:128 * (k + 1)])

        in_maps.append({
            "w0s": w0s, "w1s": w1s, "wcs": wcs, "wot": wot, "wpt": wpt,
            "enct": enct, "encse": encse, "wordt": wordt,
            "h0t": h0t, "c0s": c0s,
        })
    return in_maps


_CACHE = {}


def _get_nc(t_steps=T):
    if t_steps not in _CACHE:
        _CACHE[t_steps] = _build(t_steps)
    return _CACHE[t_steps]


def run_device(inputs, trace=False, t_steps=T):
    nc = _get_nc(t_steps)
    in_maps = _prep(inputs, t_steps)
    return run_bass_kernel_spmd(nc, in_maps, core_ids=list(range(P)), trace=trace)


def assemble(results, t_steps=T):
    return np.concatenate(
        [np.ascontiguousarray(np.asarray(results[k]["out"]).astype(np.float32).T)
         .reshape(t_steps, B, VSH) for k in range(P)],
        axis=2)


def kernel(**inputs):
    r = run_device(inputs)
    return assemble(r.results)


# revision 20
# speedup vs baseline: 1.6007x; 1.0249x over previous
"""Trainium2 Bass kernel for nn_Decoder (LSTM decoder w/ attention).

Sharding: 8-way model parallel over hidden dim D for the recurrence
(each core owns 128 of 1024 dims = all 4 gates for those dims), vocab
shard (4000 rows/core) for the output projection, which runs as a
batched matmul over all T*B rows interleaved with the recurrence.

All matmul operands are bf16 (1 cycle/row on the PE vs 4 for fp32);
accumulation stays fp32 in PSUM, LSTM cell state and softmax stay fp32.
Collective payloads and the final score store are bf16 (host casts back).

Perf-critical details (learned from NTFF traces):
- Collective input stagings must be contiguous >=32B segments: sub-32B
  scattered HBM writes trigger read-modify-write and delay the DMA
  completion semaphore by ~20us, stalling the next collective trigger.
  Logits are therefore computed directly in [b, s] orientation and the
  AllReduce buffers keep the natural [128, 256] layout.
- Projection chunks (width 256) are interleaved into all three
  per-step collective flights to keep the PE busy (p-state) and to
  absorb the ~24us/step of collective latency.
- Projection stores issue from the Scalar HWDGE queue so they never
  head-of-line block the Sync queue carrying recurrence DMAs.
"""

import numpy as np
import ml_dtypes
import bass_rust
import concourse.bass as bass  # noqa: F401  (bass types used via bacc)
import concourse.tile as tile
from concourse import bacc, mybir
from concourse.bass_utils import run_bass_kernel_spmd
from concourse.masks import make_identity

V, E, D = 32000, 512, 1024
TWO_E = 1024
B, S, T = 32, 64, 48
P = 8
DSH = D // P        # 128 hidden dims per core
VSH = V // P        # 4000 vocab rows per core
FP = mybir.dt.float32
BF = mybir.dt.bfloat16
NBF = ml_dtypes.bfloat16
AF = mybir.ActivationFunctionType
RG = [list(range(P))]
X = mybir.AxisListType.X

# gates0 lhsT layout: [h2 (8x128) | word (4x128) | ones/bias (128) | av (8x128)]
NK0 = 21
# gates1 lhsT layout: [h1 (8x128) | ones/bias (128)]
NK1 = 9

PW = 256            # projection chunk width (cols = t*32+b)


def _build(t_steps=T):
    nc = bacc.Bacc("TRN2", target_bir_lowering=False, debug=False, num_devices=P)
    CW = t_steps * 32  # avhist block width (cols = t*32+b)

    w0s_p = nc.declare_dram_parameter("w0s", [128, NK0 * 512], BF, isOutput=False)
    w1s_p = nc.declare_dram_parameter("w1s", [128, NK1 * 512], BF, isOutput=False)
    wcs_p = nc.declare_dram_parameter("wcs", [128, 16 * 128], BF, isOutput=False)
    wot_p = nc.declare_dram_parameter("wot", [1024, VSH], BF, isOutput=False)
    wpt_p = nc.declare_dram_parameter("wpt", [128, 8 * 128], BF, isOutput=False)
    enct_p = nc.declare_dram_parameter("enct", [1024, 2048], BF, isOutput=False)
    encse_p = nc.declare_dram_parameter("encse", [64, 32 * 128], BF, isOutput=False)
    wordt_p = nc.declare_dram_parameter("wordt", [128, t_steps * 128], BF,
                                        isOutput=False)
    h0t_p = nc.declare_dram_parameter("h0t", [128, 8 * 32], BF, isOutput=False)
    c0s_p = nc.declare_dram_parameter("c0s", [32, 128], FP, isOutput=False)
    # scores stored transposed: [vocab_shard, t*32+b]
    out_p = nc.declare_dram_parameter("out", [VSH, CW], BF, isOutput=True)

    with tile.TileContext(nc) as tc:
        with (
            tc.tile_pool(name="res", bufs=1) as res,
            tc.tile_pool(name="wk", bufs=2) as wk,
            tc.tile_pool(name="ps1", bufs=1, space="PSUM") as ps1,
            tc.tile_pool(name="ps2", bufs=2, space="PSUM") as ps2,
            tc.tile_pool(name="dr", bufs=2, space="DRAM") as dr,
        ):
            # ---- resident SBUF ----
            w0s = res.tile([128, NK0 * 512], BF, tag="w0s")
            w1s = res.tile([128, NK1 * 512], BF, tag="w1s")
            wcs = res.tile([128, 16 * 128], BF, tag="wcs")
            at = res.tile([128, 2048], BF, tag="at")
            encse = res.tile([64, 32 * 128], BF, tag="encse")
            avhist = res.tile([128, 8 * CW], BF, tag="avhist")
            wotsb = res.tile([128, 8 * VSH], BF, tag="wotsb")
            wordsb = res.tile([128, t_steps * 128], BF, tag="wordsb")
            h1full = res.tile([128, 8 * 32], BF, tag="h1full")
            h2full = res.tile([128, 8 * 32], BF, tag="h2full")
            c = res.tile([32, 128], FP, tag="c")
            ones = res.tile([128, 32], BF, tag="ones")
            id32 = res.tile([32, 32], FP, tag="id32")
            id64 = res.tile([64, 64], FP, tag="id64")
            wpt = res.tile([128, 8 * 128], BF, tag="wpt")

            # ---- init loads (split for overlap) ----
            for kk in range(NK0):
                nc.sync.dma_start(out=w0s[:, 512 * kk:512 * (kk + 1)],
                                  in_=w0s_p[:, 512 * kk:512 * (kk + 1)])
            for kk in range(NK1):
                nc.sync.dma_start(out=w1s[:, 512 * kk:512 * (kk + 1)],
                                  in_=w1s_p[:, 512 * kk:512 * (kk + 1)])
            nc.sync.dma_start(out=wcs[:], in_=wcs_p[:])
            nc.sync.dma_start(out=encse[:], in_=encse_p[:])
            nc.sync.dma_start(out=h2full[:], in_=h0t_p[:])
            nc.sync.dma_start(out=c[:], in_=c0s_p[:])
            nc.sync.dma_start(out=wpt[:], in_=wpt_p[:])
            nc.sync.dma_start(out=wordsb[:], in_=wordt_p[:])
            for j in range(8):
                nc.scalar.dma_start(
                    out=wotsb[:, VSH * j:VSH * (j + 1)],
                    in_=wot_p[128 * j:128 * (j + 1), :])

            nc.vector.memset(ones[:], 0.0)
            nc.vector.memset(ones[0:1, :], 1.0)
            make_identity(nc, id32[:])
            make_identity(nc, id64[:])

            # ---- attention scores AT_shard = Wp_shard @ encT ----
            at_ps = [
                ps2.tile([128, 512], FP, tag="mm", name="atps_0"),
                ps2.tile([128, 512], FP, tag="mm", name="atps_1"),
                ps2.tile([128, 512], FP, tag="tr", name="atps_2"),
                ps1.tile([128, 512], FP, tag="av", name="atps_3"),
            ]
            for kk in range(8):
                et = wk.tile([128, 2048], BF, tag="enct", bufs=1)
                nc.sync.dma_start(out=et[:], in_=enct_p[128 * kk:128 * (kk + 1), :])
                for nch in range(4):
                    nc.tensor.matmul(at_ps[nch][:],
                                     wpt[:, 128 * kk:128 * (kk + 1)],
                                     et[:, 512 * nch:512 * (nch + 1)],
                                     start=(kk == 0), stop=(kk == 7))
            for nch in range(4):
                nc.scalar.activation(at[:, 512 * nch:512 * (nch + 1)],
                                     at_ps[nch][:], AF.Copy)

            # output projection (transposed): outT[v, (t,b)] += WoT.T @ av
            # chunk list: (col base, width, vt); the final 256-col group is
            # split in two width-128 halves so only the last half (needing
            # the final av column) remains for the tail.
            CHUNKS = []
            nfull = CW // PW - 1
            for n8 in range(nfull):
                for vt in range(32):
                    CHUNKS.append((PW * n8, PW, vt))
            for half in range(2):
                for vt in range(32):
                    CHUNKS.append((PW * nfull + 128 * half, 128, vt))
            kctr = [0]
            combine_last = [None]

            def _emit_pb(k):
                base, width, vt = CHUNKS[k]
                mv = min(128, VSH - 128 * vt)
                bp = ps2.tile([mv, width], FP, tag="g0", name=f"pb_{k}")
                for j in range(8):
                    nc.tensor.matmul(
                        bp[:], wotsb[:, VSH * j + 128 * vt:VSH * j + 128 * vt + mv],
                        avhist[:, j * CW + base:j * CW + base + width],
                        start=(j == 0), stop=(j == 7))
                bs_ = wk.tile([mv, width], BF, tag="bstg", name=f"pbs_{k}")
                nc.vector.tensor_copy(bs_[:], bp[:])
                nc.scalar.dma_start(
                    out=out_p[128 * vt:128 * vt + mv, base:base + width],
                    in_=bs_[:])

            def _proj_slot(t, max_chunks=2):
                done = 0
                while done < max_chunks and kctr[0] < len(CHUNKS):
                    base, width, vt = CHUNKS[kctr[0]]
                    if base + width > 32 * t:
                        break
                    _emit_pb(kctr[0])
                    kctr[0] += 1
                    done += 1

            # ---- recurrence ----
            for t in range(t_steps):
                # gates0 part 1: h2 / word / bias contributions (fills the
                # AR(t-1) flight on the PE; h2full(t-1) is loaded right
                # after AG(bm, t-1) completes)
                g0 = ps2.tile([32, 512], FP, tag="g0")
                mms = []
                for j in range(8):
                    mms.append((h2full[:, 32 * j:32 * (j + 1)], j))
                for j in range(4):
                    mms.append((wordsb[:, 128 * t + 32 * j:128 * t + 32 * (j + 1)],
                                8 + j))
                mms.append((ones[:], 12))
                for i, (lhsT, kk) in enumerate(mms):
                    mm = nc.tensor.matmul(g0[:], lhsT,
                                          w0s[:, 512 * kk:512 * (kk + 1)],
                                          start=(i == 0),
                                          stop=(t == 0 and i == len(mms) - 1))
                    if i == 0 and combine_last[0] is not None:
                        # keep the PE from hoisting next-step gate matmuls
                        # ahead of the time-critical softmax->AR path
                        bass_rust.add_dep_helper(
                            mm.ins, combine_last[0].ins, sync=False,
                            reason="g0 part1 after combine")

                # av(t-1): load summed avpre from AR output (two pipelined
                # halves), tanh into avhist column t-1, then accumulate the
                # av part of gates0
                if t > 0:
                    avpre = wk.tile([128, 256], BF, tag="avpre")
                    avh_r = avhist[:].rearrange("p (j c) -> p j c", c=CW)
                    for half in range(2):
                        hs = slice(128 * half, 128 * (half + 1))
                        nc.sync.dma_start(out=avpre[:, hs], in_=oav_prev[:, hs])
                        dst = avh_r[:, 4 * half:4 * (half + 1),
                                    32 * (t - 1):32 * t]
                        nc.scalar.activation(
                            dst,
                            avpre[:, hs].rearrange("p (j b) -> p j b", b=32),
                            AF.Tanh)
                        for j in range(4 * half, 4 * half + 4):
                            nc.tensor.matmul(
                                g0[:],
                                avhist[:, j * CW + 32 * (t - 1):j * CW + 32 * t],
                                w0s[:, 512 * (13 + j):512 * (14 + j)],
                                start=False, stop=(j == 7))
                # t==0: av contribution omitted (prev context is zeros in
                # the reference init); accumulation closed in part 1.

                # lstm cell 0 (gate order i,f,o,g along free dim)
                sifo = wk.tile([32, 384], FP, tag="sifo")
                tg = wk.tile([32, 128], FP, tag="tg")
                nc.scalar.activation(sifo[:], g0[:, 0:384], AF.Sigmoid)
                nc.scalar.activation(tg[:], g0[:, 384:512], AF.Tanh)
                t1 = wk.tile([32, 128], FP, tag="t1")
                t2 = wk.tile([32, 128], FP, tag="t2")
                nc.vector.tensor_mul(t1[:], sifo[:, 128:256], c[:])
                nc.vector.tensor_mul(t2[:], sifo[:, 0:128], tg[:])
                nc.vector.tensor_add(c[:], t1[:], t2[:])
                tc1 = wk.tile([32, 128], FP, tag="tc1")
                nc.scalar.activation(tc1[:], c[:], AF.Tanh)
                h1 = wk.tile([32, 128], FP, tag="h1")
                nc.vector.tensor_mul(h1[:], sifo[:, 256:384], tc1[:])

                # h1 -> h1T shard, AllGather -> h1full
                trp = ps2.tile([128, 32], FP, tag="tr")
                nc.tensor.transpose(trp[:], h1[:], id32[:])
                h1t = wk.tile([128, 32], BF, tag="h1t")
                nc.scalar.activation(h1t[:], trp[:], AF.Copy)
                b1 = dr.tile([128, 32], BF, tag="b1")
                o1 = dr.tile([1024, 32], BF, tag="o1", addr_space="Shared")
                nc.sync.dma_start(out=b1[:], in_=h1t[:])
                nc.gpsimd.collective_compute(
                    "AllGather", mybir.AluOpType.bypass,
                    replica_groups=RG, ins=[b1.opt()], outs=[o1.opt()])

                # PE filler during AG(h1) flight
                _proj_slot(t)

                # split into 8 per-chunk DMAs across both HWDGE queues so
                # gates1 matmul j can start as soon as its chunk lands
                for j in range(8):
                    eng = nc.sync if j % 2 == 0 else nc.scalar
                    eng.dma_start(out=h1full[:, 32 * j:32 * (j + 1)],
                                  in_=o1[128 * j:128 * (j + 1), :])

                # gates1: g1 = W1 @ [h1; 1]
                g1 = ps2.tile([32, 512], FP, tag="mm")
                for j in range(8):
                    nc.tensor.matmul(g1[:], h1full[:, 32 * j:32 * (j + 1)],
                                     w1s[:, 512 * j:512 * (j + 1)],
                                     start=(j == 0), stop=False)
                nc.tensor.matmul(g1[:], ones[:], w1s[:, 512 * 8:512 * 9],
                                 start=False, stop=True)

                # lstm cell 1 (same c state threaded; gate order i,f,o,g)
                sifo2 = wk.tile([32, 384], FP, tag="sifo")
                tg2 = wk.tile([32, 128], FP, tag="tg")
                nc.scalar.activation(sifo2[:], g1[:, 0:384], AF.Sigmoid)
                nc.scalar.activation(tg2[:], g1[:, 384:512], AF.Tanh)
                nc.vector.tensor_mul(t1[:], sifo2[:, 128:256], c[:])
                nc.vector.tensor_mul(t2[:], sifo2[:, 0:128], tg2[:])
                nc.vector.tensor_add(c[:], t1[:], t2[:])
                nc.scalar.activation(tc1[:], c[:], AF.Tanh)
                h2 = wk.tile([32, 128], FP, tag="h2")
                nc.vector.tensor_mul(h2[:], sifo2[:, 256:384], tc1[:])

                # h2 -> h2T shard
                trp2 = ps2.tile([128, 32], FP, tag="tr")
                nc.tensor.transpose(trp2[:], h2[:], id32[:])
                h2t = wk.tile([128, 32], BF, tag="h2t")
                nc.scalar.activation(h2t[:], trp2[:], AF.Copy)

                # logits partial per-batch ([s, b]), then transpose to
                # [b, s] so the bm staging write is 32 contiguous 128B
                # segments (sub-32B scattered HBM writes are catastrophic)
                lgps = ps2.tile([64, 32], FP, tag="tr", name="lgps")
                for b in range(32):
                    nc.tensor.matmul(lgps[:, b:b + 1],
                                     at[:, 64 * b:64 * (b + 1)],
                                     h2t[:, b:b + 1],
                                     start=True, stop=True)
                lgf = wk.tile([64, 32], FP, tag="lgf")
                nc.vector.tensor_copy(lgf[:], lgps[:])
                lgt_ps = ps2.tile([32, 64], FP, tag="tr", name="lgt_ps")
                nc.tensor.transpose(lgt_ps[:], lgf[:], id64[:])
                lgc = wk.tile([32, 64], BF, tag="lgc")
                nc.vector.tensor_copy(lgc[:], lgt_ps[:])

                # one merged AllGather: [h2T shard (128x32) | logits (b-major)]
                bm = dr.tile([192, 32], BF, tag="bm")
                om = dr.tile([1536, 32], BF, tag="om", addr_space="Shared")
                nc.sync.dma_start(out=bm[0:128, :], in_=h2t[:])
                bm_ap = bm[:]
                # flat bm index for logit (b, s) is 128*32 + 64b + s; lgc is
                # [b, s] so this is 32 contiguous 128B rows
                bm_lp = bass_rust.AP(bm_ap.tensor, bm_ap.offset + 128 * 32,
                                     [[64, 32], [1, 64]])
                nc.sync.dma_start(out=bm_lp, in_=lgc[:])
                nc.gpsimd.collective_compute(
                    "AllGather", mybir.AluOpType.bypass,
                    replica_groups=RG, ins=[bm.opt()], outs=[om.opt()])

                # PE filler during AG(bm) flight
                _proj_slot(t)

                om_ap = om[:]
                # ls first (softmax is on the critical path; h2full only
                # feeds next-step gates which have slack)
                # ls[b, r*64+s] = om[r*192 + 128 + (64b+s)//32, (64b+s)%32]
                ls = wk.tile([32, 8 * 64], BF, tag="ls")
                nc.sync.dma_start(
                    out=ls[:].rearrange("p (r s) -> p r s", s=64),
                    in_=bass_rust.AP(om_ap.tensor, om_ap.offset + 128 * 32,
                                     [[64, 32], [192 * 32, 8], [1, 64]]))
                # h2full[p, j*32+b] = om[j*192 + p, b]
                nc.scalar.dma_start(
                    out=h2full[:].rearrange("p (j b) -> p j b", b=32),
                    in_=bass_rust.AP(om_ap.tensor, om_ap.offset,
                                     [[32, 128], [192 * 32, 8], [1, 32]]))
                # tree sum of the 8 rank partials
                lv1 = wk.tile([32, 256], FP, tag="lv1")
                lv2 = wk.tile([32, 128], FP, tag="lv2")
                lg0 = wk.tile([32, 64], FP, tag="lgs")
                nc.vector.tensor_add(lv1[:], ls[:, 0:256], ls[:, 256:512])
                nc.vector.tensor_add(lv2[:], lv1[:, 0:128], lv1[:, 128:256])
                nc.vector.tensor_add(lg0[:], lv2[:, 0:64], lv2[:, 64:128])

                # softmax over s
                mx = wk.tile([32, 1], FP, tag="mx")
                nc.vector.reduce_max(mx[:], lg0[:], axis=X, negate=True)
                ex = wk.tile([32, 64], FP, tag="ex")
                nc.scalar.activation(ex[:], lg0[:], AF.Exp, bias=mx[:])
                sm = wk.tile([32, 1], FP, tag="sm")
                nc.vector.reduce_sum(sm[:], ex[:], axis=X)
                rc = wk.tile([32, 1], FP, tag="rc")
                nc.vector.reciprocal(rc[:], sm[:])
                al = wk.tile([32, 64], FP, tag="al")
                nc.vector.tensor_scalar_mul(al[:], ex[:], rc[:])

                # alpha -> alphaT
                trp3 = ps2.tile([64, 32], FP, tag="tr")
                nc.tensor.transpose(trp3[:], al[:], id32[:])
                alt = wk.tile([64, 32], BF, tag="alt")
                nc.scalar.activation(alt[:], trp3[:], AF.Copy)

                # context per-batch (transposed): cxt[:, b] = encse_b.T @ alpha_b
                cxt_ps = ps2.tile([128, 32], FP, tag="tr")
                for b in range(32):
                    nc.tensor.matmul(cxt_ps[:, b:b + 1],
                                     encse[:, 128 * b:128 * (b + 1)],
                                     alt[:, b:b + 1],
                                     start=True, stop=True)
                cxt = wk.tile([128, 32], BF, tag="cxt")
                nc.scalar.activation(cxt[:], cxt_ps[:], AF.Copy)

                # combine partial: av_preT[m-dims, b] over own 256 K dims
                avp = ps1.tile([128, 256], FP, tag="av")
                for m in range(8):
                    nc.tensor.matmul(avp[:, 32 * m:32 * (m + 1)],
                                     wcs[:, (0 * 8 + m) * 128:(0 * 8 + m) * 128 + 128],
                                     h2t[:], start=True, stop=False)
                    combine_last[0] = nc.tensor.matmul(
                        avp[:, 32 * m:32 * (m + 1)],
                        wcs[:, (1 * 8 + m) * 128:(1 * 8 + m) * 128 + 128],
                        cxt[:], start=False, stop=True)
                avs = wk.tile([128, 256], BF, tag="avs")
                nc.vector.tensor_copy(avs[:], avp[:])
                # AllReduce in the native [128, 256] layout: staging and
                # readback are fully contiguous per partition (512B segs)
                bav = dr.tile([128, 256], BF, tag="bav")
                oav = dr.tile([128, 256], BF, tag="oav", addr_space="Shared")
                nc.sync.dma_start(out=bav[:], in_=avs[:])
                nc.gpsimd.collective_compute(
                    "AllReduce", mybir.AluOpType.add,
                    replica_groups=RG, ins=[bav.opt()], outs=[oav.opt()])
                oav_prev = oav

                # PE filler during AR flight
                _proj_slot(t)

            # ---- final av column + remaining projection chunks ----
            avpre = wk.tile([128, 256], BF, tag="avpre")
            nc.sync.dma_start(out=avpre[:], in_=oav_prev[:])
            dst = (avhist[:].rearrange("p (j c) -> p j c", c=CW)
                   [:, :, 32 * (t_steps - 1):32 * t_steps])
            nc.scalar.activation(
                dst, avpre[:].rearrange("p (j b) -> p j b", b=32), AF.Tanh)
            while kctr[0] < len(CHUNKS):
                _emit_pb(kctr[0])
                kctr[0] += 1

    nc.compile()
    return nc


def _prep(inputs, t_steps=T):
    g = {k: np.asarray(v) for k, v in inputs.items()}
    src = g["src_encodings"].astype(np.float32)          # [S, B, 2E]
    h0 = g["h0"].astype(np.float32)
    c0 = g["c0"].astype(np.float32)
    emb = g["embedding"].astype(np.float32)
    Wp = g["W_proj"].astype(np.float32)
    Wc = g["W_combine"].astype(np.float32)
    Wo = g["W_out"].astype(np.float32)
    Wih0 = g["W_ih0"].astype(np.float32)
    Whh0 = g["W_hh0"].astype(np.float32)
    bih0 = g["b_ih0"].astype(np.float32)
    bhh0 = g["b_hh0"].astype(np.float32)
    Wih1 = g["W_ih1"].astype(np.float32)
    Whh1 = g["W_hh1"].astype(np.float32)
    bih1 = g["b_ih1"].astype(np.float32)
    bhh1 = g["b_hh1"].astype(np.float32)
    tgt = np.asarray(g["tgt_tensor"]).astype(np.int64)   # [T, B]

    W1 = Wih1 + Whh1
    b0 = bih0 + bhh0
    b1 = bih1 + bhh1

    # shared across cores
    wemb = emb[tgt[:t_steps]]                            # [t, B, E]
    # wordt: [128, t*128]; step block t = wordT[:,t] split into 4 j-blocks
    wordt = (wemb.transpose(0, 2, 1)                     # [t, E, B]
             .reshape(t_steps, 4, 128, 32)
             .transpose(2, 0, 1, 3).reshape(128, t_steps * 128))
    wordt = np.ascontiguousarray(wordt).astype(NBF)
    enct = np.ascontiguousarray(
        src.transpose(2, 1, 0).reshape(1024, 2048)).astype(NBF)  # [e, b*64+s]
    h0t = np.ascontiguousarray(
        h0.T.reshape(8, 128, 32).transpose(1, 0, 2).reshape(128, 256)).astype(NBF)

    in_maps = []
    for k in range(P):
        rows = np.concatenate([gg * 1024 + k * 128 + np.arange(128)
                               for gg in (0, 1, 3, 2)])  # [i|f|o|g] x 128 dims
        # W0sT_aug rows: [h2 1024 | word 512 | bias 1 | pad | av 1024]
        w0a = np.zeros((NK0 * 128, 512), np.float32)
        w0a[0:1024] = Whh0[rows].T
        w0a[1024:1536] = Wih0[rows, 0:512].T
        w0a[1536] = b0[rows]
        w0a[1664:2688] = Wih0[rows, 512:1536].T
        w0s = np.ascontiguousarray(
            w0a.reshape(NK0, 128, 512).transpose(1, 0, 2)
            .reshape(128, NK0 * 512)).astype(NBF)

        w1a = np.zeros((NK1 * 128, 512), np.float32)
        w1a[0:1024] = W1[rows].T
        w1a[1024] = b1[rows]
        w1s = np.ascontiguousarray(
            w1a.reshape(NK1, 128, 512).transpose(1, 0, 2)
            .reshape(128, NK1 * 512)).astype(NBF)

        # Wc own-K slice: h dims [128k..] and ctx dims [1024+128k..]
        hs = slice(k * 128, k * 128 + 128)
        cs = slice(1024 + k * 128, 1024 + k * 128 + 128)
        wc_own = np.concatenate([Wc[:, hs], Wc[:, cs]], axis=1)  # [1024, 256]
        blocks = []
        for j in range(2):
            for m in range(8):
                blocks.append(wc_own[128 * m:128 * (m + 1),
                                     128 * j:128 * (j + 1)].T)
        wcs = np.ascontiguousarray(np.concatenate(blocks, axis=1)).astype(NBF)

        wot = np.ascontiguousarray(Wo[VSH * k:VSH * (k + 1)].T).astype(NBF)
        wpt_ = Wp[128 * k:128 * (k + 1), :].T                       # [1024, 128]
        wpt = np.ascontiguousarray(
            wpt_.reshape(8, 128, 128).transpose(1, 0, 2)
            .reshape(128, 8 * 128)).astype(NBF)
        # encse2[s, b*128 + e] = src[s, b, e_shard]
        encse = np.ascontiguousarray(
            src[:, :, 128 * k:128 * (k + 1)].reshape(64, 32 * 128)).astype(NBF)
        c0s = np.ascontiguousarray(c0[:, 128 * k:128 * (k + 1)])

        in_maps.append({
            "w0s": w0s, "w1s": w1s, "wcs": wcs, "wot": wot, "wpt": wpt,
            "enct": enct, "encse": encse, "wordt": wordt,
            "h0t": h0t, "c0s": c0s,
        })
    return in_maps


_CACHE = {}


def _get_nc(t_steps=T):
    if t_steps not in _CACHE:
        _CACHE[t_steps] = _build(t_steps)
    return _CACHE[t_steps]


def run_device(inputs, trace=False, t_steps=T):
    nc = _get_nc(t_steps)
    in_maps = _prep(inputs, t_steps)
    return run_bass_kernel_spmd(nc, in_maps, core_ids=list(range(P)), trace=trace)


def assemble(results, t_steps=T):
    return np.concatenate(
        [np.ascontiguousarray(np.asarray(results[k]["out"]).astype(np.float32).T)
         .reshape(t_steps, B, VSH) for k in range(P)],
        axis=2)


def kernel(**inputs):
    r = run_device(inputs)
    return assemble(r.results)
